# revision 1
# baseline (speedup 1.0000x reference)
# Trainium2 Bass kernel for nn_BDH_66056597013022 (dense_transformer).
#
# Model (per reference):
#   v = LN(emb_w[tokens])                                  [B,T,D]
#   6x: x  = relu(v @ Dx_h)            per head            [B,H,T,Dh]
#       xr = RoPE(x)
#       S  = xr @ xr^T                 (no softmax)        [B,H,T,T]
#       a  = S @ v                                         [B,H,T,D]
#       y  = relu(a @ Dy_h) * x                            [B,H,T,Dh]
#       v  = LN(v + LN(concat_h(y) @ E))
#   out = v @ readout                                      [B,T,V]
#
# Shapes: B=4 T=1024 H=4 N=4096 D=256 L=6 V=256, Dh=N/H=1024.
#
# Sharding (8 cores): core c -> batch b=c//2, head-pair hp=c%2 (heads 2hp,2hp+1).
# All per-head work is local; the only cross-core coupling is the head-sum in
# z = y @ E, handled with a 2-rank AllReduce per layer between cores {2b,2b+1}.
# Both cores of a pair then redundantly compute the LN/v-update, so the whole
# forward stays on-device; even cores' outputs are returned.
#
# On-chip layouts per core (SBUF):
#   v    [T,D]   8 tiles [128,256]   (token rows on partitions)
#   vT   [D,T]   2 tiles [128,1024]  (for contractions over D)
#   xT,xrT,yT [Dh,T] 8 tiles [128,1024] each head (Dh on partitions)
#   S streamed per 128-row block [128,1024]; aT [D,T] 2 tiles.
# All matmuls are out = lhsT.T @ rhs with K<=128 on partitions; S is
# numerically symmetric so its [t,s] tiles serve as [s,t] operands directly.
#
# Matmul operands are float32r (TF32-like: ~1e-4 rounding, 1 cycle/row at
# N>=256 vs 4 for fp32). f32r operands must be produced by a compute
# instruction that rounds (ACT/DVE write with f32r out dtype); DMA-produced
# weights get a one-time ACT round-copy. PSUM accumulation stays fp32.

import os
import numpy as np

B, T, H, N, D, L, V = 4, 1024, 4, 4096, 256, 6, 256
Dh = N // H
EPS = 1e-5
NCORES = 8
P = 128
NT = T // P  # 8 tiles of tokens
ND = D // P  # 2 tiles of model dim
NDh = Dh // P  # 8 tiles of head dim

_CACHE = {}
LAST_RESULT = None


def _build_program():
    from contextlib import ExitStack

    import concourse.bass as bass
    import concourse.bacc as bacc
    import concourse.tile as tile
    import concourse.mybir as mybir
    from concourse.masks import make_identity

    f32 = mybir.dt.float32
    f32r = mybir.dt.float32r
    AF = mybir.ActivationFunctionType
    ALU = mybir.AluOpType
    ts = bass.ts

    nc = bacc.Bacc("TRN2", target_bir_lowering=False, debug=False,
                   enable_asserts=False, num_devices=NCORES)

    d_oh = nc.dram_tensor("onehotT", [V, T], f32, kind="ExternalInput").ap()
    d_ew = nc.dram_tensor("emb_w", [V, D], f32, kind="ExternalInput").ap()
    d_dx = nc.dram_tensor("dx", [2 * D, Dh], f32, kind="ExternalInput").ap()
    d_dy = nc.dram_tensor("dy", [2 * D, Dh], f32, kind="ExternalInput").ap()
    d_eh = nc.dram_tensor("eh", [2 * Dh, D], f32, kind="ExternalInput").ap()
    d_cos = nc.dram_tensor("cosT", [Dh // 2, T], f32, kind="ExternalInput").ap()
    d_sin = nc.dram_tensor("sinT", [Dh // 2, T], f32, kind="ExternalInput").ap()
    d_ro = nc.dram_tensor("readout", [D, V], f32, kind="ExternalInput").ap()
    d_out = nc.dram_tensor("out", [T, V], f32, kind="ExternalOutput").ap()

    with tile.TileContext(nc) as tc, ExitStack() as ctx:
        wpool = ctx.enter_context(tc.tile_pool(name="weights", bufs=1))
        vpool = ctx.enter_context(tc.tile_pool(name="vpool", bufs=1))
        xpool = ctx.enter_context(tc.tile_pool(name="xpool", bufs=8))
        ehpool = ctx.enter_context(tc.tile_pool(name="ehpool", bufs=3))
        xrpool = ctx.enter_context(tc.tile_pool(name="xrpool", bufs=8))
        spool = ctx.enter_context(tc.tile_pool(name="spool", bufs=2))
        apool = ctx.enter_context(tc.tile_pool(name="apool", bufs=2))
        ypool = ctx.enter_context(tc.tile_pool(name="ypool", bufs=3))
        zpool = ctx.enter_context(tc.tile_pool(name="zpool", bufs=1))
        lnpool = ctx.enter_context(tc.tile_pool(name="lnpool", bufs=3))
        stpool = ctx.enter_context(tc.tile_pool(name="stpool", bufs=4))
        rtpool = ctx.enter_context(tc.tile_pool(name="rtpool", bufs=2))
        psA = ctx.enter_context(tc.tile_pool(name="psA", bufs=2, space="PSUM"))
        psB = ctx.enter_context(tc.tile_pool(name="psB", bufs=2, space="PSUM"))
        dpool = ctx.enter_context(tc.tile_pool(name="drampool", bufs=2, space="DRAM"))

        # ---- persistent weights (DMA to staging, then round-copy to f32r) ----
        # Staging cycles through several big pools so the DMAs and round
        # copies pipeline instead of ping-ponging through two slots.
        _stage_slots = [(rtpool, "ropetmp"), (ypool, "yT"), (apool, "aT"),
                        (spool, "score"), (rtpool, "ropetmp"), (ypool, "yT"),
                        (apool, "aT"), (ypool, "yT")]
        _stage_i = [0]

        def load_rounded(dram_ap, n_tiles, width, tag):
            tiles = []
            for i in range(n_tiles):
                pool, ptag = _stage_slots[_stage_i[0] % len(_stage_slots)]
                _stage_i[0] += 1
                stg = pool.tile([P, T], f32, tag=ptag, name=f"stg_{tag}{i}")
                nc.sync.dma_start(stg[:, :width], dram_ap[ts(i, P), :])
                wt = wpool.tile([P, width], f32r, tag=f"{tag}{i}", name=f"{tag}{i}")
                nc.scalar.copy(wt[:], stg[:, :width])
                tiles.append(wt)
            return tiles

        dx_sb = load_rounded(d_dx, 4, Dh, "dx")
        dy_sb = load_rounded(d_dy, 4, Dh, "dy")

        def stream_rounded(dram_ap, i, width, name):
            # eh/ew/ro are streamed from HBM on demand (frees ~18KB SBUF)
            stg = rtpool.tile([P, T], f32, tag="ropetmp", name=f"stg_{name}")
            nc.sync.dma_start(stg[:, :width], dram_ap[ts(i, P), :])
            rt = ehpool.tile([P, width], f32r, tag="ehr", name=name)
            nc.scalar.copy(rt[:], stg[:, :width])
            return rt
        cos_sb = []
        sin_sb = []
        for i in range(4):
            ct = wpool.tile([P, T], f32, tag=f"cos{i}", name=f"cos{i}")
            nc.sync.dma_start(ct[:], d_cos[ts(i, P), :])
            cos_sb.append(ct)
        for i in range(4):
            st = wpool.tile([P, T], f32, tag=f"sin{i}", name=f"sin{i}")
            nc.sync.dma_start(st[:], d_sin[ts(i, P), :])
            sin_sb.append(st)
        ident = wpool.tile([P, P], f32, tag="ident", name="ident")
        make_identity(nc, ident)
        identr = wpool.tile([P, P], f32r, tag="identr", name="identr")
        nc.scalar.copy(identr[:], ident[:])
        epsc = wpool.tile([P, 1], f32, tag="epsc", name="epsc")
        nc.gpsimd.memset(epsc[:], EPS)
        warmsink = wpool.tile([P, 1], f32, tag="warmsink", name="warmsink")

        def keep_pe_warm(n_mms, label):
            # HAM re-throttles the PE to 1.2 GHz after ~3.4us of idle; during
            # known stall windows (RoPE on DVE, AllReduce in flight) feed the
            # PE dependency-free matmuls so the clock stays at 2.4 GHz.
            wps = psA.tile([P, 512], f32, tag="psA", name=f"warm_{label}")
            for i in range(n_mms):
                nc.tensor.matmul(wps[:], dx_sb[0][:, 0:P], dx_sb[1][:, 0:512],
                                 start=(i == 0), stop=(i == n_mms - 1))
            nc.scalar.copy(warmsink[:], wps[:, 0:1])

        # ---- persistent activations ----
        v_sb = [vpool.tile([P, D], f32r, tag=f"v{m}", name=f"v{m}")
                for m in range(NT)]
        vT_sb = [vpool.tile([P, T], f32r, tag=f"vT{k}", name=f"vT{k}")
                 for k in range(ND)]

        def layer_norm(src_ap, dst_ap):
            st6 = stpool.tile([P, 6], f32, tag="st6", name="st6")
            nc.vector.bn_stats(st6[:], src_ap)
            mv = stpool.tile([P, 2], f32, tag="mv", name="mv")
            nc.vector.bn_aggr(mv[:], st6[:])
            sd = stpool.tile([P, 1], f32, tag="sd", name="sd")
            nc.scalar.activation(sd[:], mv[:, 1:2], AF.Sqrt, bias=epsc[:], scale=1.0)
            rstd = stpool.tile([P, 1], f32, tag="rstd", name="rstd")
            nc.vector.reciprocal(rstd[:], sd[:])
            nmr = stpool.tile([P, 1], f32, tag="nmr", name="nmr")
            nc.vector.scalar_tensor_tensor(
                nmr[:], mv[:, 0:1], -1.0, rstd[:], op0=ALU.mult, op1=ALU.mult)
            nc.scalar.activation(dst_ap, src_ap, AF.Identity,
                                 bias=nmr[:], scale=rstd[:])

        def transpose_v():
            # vT[d, t] <- v[t, d]; f32r transpose (1.5 cyc/row vs 4 for the
            # fp32 two-pass), the ACT eviction rounds into the f32r vT tile.
            for m in range(NT):
                for d in range(ND):
                    tps = psA.tile([P, P], f32r, tag="psA", name="tps")
                    nc.tensor.transpose(
                        tps[:], v_sb[m][:, ts(d, P)], identr[:])
                    nc.scalar.copy(vT_sb[d][:, ts(m, P)], tps[:])

        # ---- embedding: v0 = LN(onehot @ emb_w) ----
        oh_sb = []
        for k in range(ND):
            stg = rtpool.tile([P, T], f32, tag="ropetmp", name=f"stg_oh{k}")
            nc.sync.dma_start(stg[:], d_oh[ts(k, P), :])
            oht = spool.tile([P, T], f32r, tag="score", name=f"oh{k}")
            nc.scalar.copy(oht[:], stg[:])
            oh_sb.append(oht)
        ew_sb = [stream_rounded(d_ew, k, D, f"ew{k}") for k in range(ND)]
        for m in range(NT):
            eps_t = psA.tile([P, D], f32, tag="psA", name="embps")
            for k in range(ND):
                nc.tensor.matmul(eps_t[:], oh_sb[k][:, ts(m, P)], ew_sb[k][:],
                                 start=(k == 0), stop=(k == ND - 1))
            emb_t = lnpool.tile([P, D], f32, tag="w", name="embt")
            nc.scalar.copy(emb_t[:], eps_t[:])
            layer_norm(emb_t[:], v_sb[m][:])
        transpose_v()

        rg = [[0, 1], [2, 3], [4, 5], [6, 7]]

        for layer in range(L):
            z_sb = [zpool.tile([P, T], f32r, tag=f"z{i}", name=f"z{i}_{layer}")
                    for i in range(2)]
            for j in range(2):  # local head index
                # ---- A: xT = relu(Dx^T @ vT), interleaved with RoPE ----
                xT = [None] * NDh
                xr = [None] * NDh
                for m in range(4):
                    pair = []
                    for mm in (m, m + 4):
                        xps = psA.tile([P, T], f32, tag="psA", name="xps")
                        for n in range(2):
                            for k in range(ND):
                                nc.tensor.matmul(
                                    xps[:, ts(n, 512)],
                                    dx_sb[2 * j + k][:, ts(mm, P)],
                                    vT_sb[k][:, ts(n, 512)],
                                    start=(k == 0), stop=(k == ND - 1))
                        xt = xpool.tile([P, T], f32, tag="xT", name=f"xT{mm}")
                        nc.scalar.activation(xt[:], xps[:], AF.Relu)
                        pair.append(xt)
                        xT[mm] = xt
                    # RoPE on the (m, m+4) pair; final DVE op rounds into the
                    # f32r xr tile (read back via fp32 bitcast for in-place op)
                    cm, sm = cos_sb[m], sin_sb[m]
                    lo, hi = pair
                    xrl = xrpool.tile([P, T], f32r, tag="xr", name=f"xr{m}")
                    xrh = xrpool.tile([P, T], f32r, tag="xr", name=f"xr{m + 4}")
                    t1 = rtpool.tile([P, T], f32, tag="ropetmp", name="rt1")
                    nc.vector.tensor_mul(t1[:], hi[:], sm[:])
                    nc.vector.tensor_mul(xrl[:], lo[:], cm[:])
                    nc.vector.tensor_sub(xrl[:], xrl[:].bitcast(f32), t1[:])
                    t2 = rtpool.tile([P, T], f32, tag="ropetmp", name="rt2")
                    nc.vector.tensor_mul(t2[:], lo[:], sm[:])
                    nc.vector.tensor_mul(xrh[:], hi[:], cm[:])
                    nc.vector.tensor_add(xrh[:], xrh[:].bitcast(f32), t2[:])
                    xr[m], xr[m + 4] = xrl, xrh
                keep_pe_warm(16, f"rope{layer}_{j}")

                # ---- C: S = xr @ xr^T streamed; aT += v^T @ S ----
                aT_ps = [psB.tile([P, T], f32, tag="psB", name=f"aTps{m}")
                         for m in range(ND)]
                # aT matmuls for tile k are emitted after the scores matmuls
                # of tile k+1, so the PE never waits on the ACT eviction.
                s_tiles = [None] * NT

                def emit_aT(k):
                    for m in range(ND):
                        for n in range(2):
                            nc.tensor.matmul(
                                aT_ps[m][:, ts(n, 512)],
                                v_sb[k][:, ts(m, P)],
                                s_tiles[k][:, ts(n, 512)],
                                start=(k == 0), stop=(k == NT - 1))

                for k in range(NT):
                    sps = psA.tile([P, T], f32, tag="psA", name="sps")
                    for n in range(2):
                        for kk in range(NDh):
                            nc.tensor.matmul(
                                sps[:, ts(n, 512)],
                                xr[kk][:, ts(k, P)],
                                xr[kk][:, ts(n, 512)],
                                start=(kk == 0), stop=(kk == NDh - 1))
                    s_sb = spool.tile([P, T], f32r, tag="score", name=f"s{k}")
                    nc.scalar.copy(s_sb[:], sps[:])
                    s_tiles[k] = s_sb
                    if k > 0:
                        emit_aT(k - 1)
                emit_aT(NT - 1)
                aT = []
                for m in range(ND):
                    at = apool.tile([P, T], f32r, tag="aT", name=f"aT{m}")
                    nc.scalar.copy(at[:], aT_ps[m][:])
                    aT.append(at)

                # ---- D/E: yT = relu(Dy^T @ aT) * xT ; zT += E_h^T @ yT ----
                # z is accumulated TRANSPOSED ([D,T]: 4 N=512 matmuls per k
                # instead of 8 N=256, and every psum group owns a full bank).
                # The z matmuls for tile k are emitted after the y matmuls of
                # tile k+1 so the PE never waits on the DVE relu*x fusion.
                z_ps = [psB.tile([P, T], f32, tag="psB", name=f"zps{i}")
                        for i in range(2)]
                y_tiles = [None] * NDh
                eh_t = [None] * NDh

                def emit_z(k):
                    for m in range(ND):
                        for n in range(2):
                            nc.tensor.matmul(
                                z_ps[m][:, ts(n, 512)],
                                eh_t[k][:, ts(m, P)],
                                y_tiles[k][:, ts(n, 512)],
                                start=(k == 0), stop=(k == NDh - 1))

                for k in range(NDh):
                    eh_t[k] = stream_rounded(d_eh, 8 * j + k, D, f"eh{k}")
                    yps = psA.tile([P, T], f32, tag="psA", name="yps")
                    for n in range(2):
                        for kk in range(ND):
                            nc.tensor.matmul(
                                yps[:, ts(n, 512)],
                                dy_sb[2 * j + kk][:, ts(k, P)],
                                aT[kk][:, ts(n, 512)],
                                start=(kk == 0), stop=(kk == ND - 1))
                    y_sb = ypool.tile([P, T], f32r, tag="yT", name=f"y{k}")
                    # y = max(yps, 0) * x   (fused relu+mul on DVE, f32r out)
                    nc.vector.scalar_tensor_tensor(
                        y_sb[:], yps[:], 0.0, xT[k][:], op0=ALU.max, op1=ALU.mult)
                    y_tiles[k] = y_sb
                    if k > 0:
                        emit_z(k - 1)
                emit_z(NDh - 1)
                if j == 0:
                    for i in range(2):
                        nc.scalar.copy(z_sb[i][:], z_ps[i][:])
                else:
                    for i in range(2):
                        nc.vector.scalar_tensor_tensor(
                            z_sb[i][:], z_ps[i][:], 0.0, z_sb[i][:].bitcast(f32),
                            op0=ALU.add, op1=ALU.add)

            # ---- boundary: transpose zT back to [T,D] (f32r transposes,
            # before the collective so the reduced result needs no further
            # reshaping), then two pipelined half-AllReduces over the core
            # pair, then the v update per half.
            zq = [spool.tile([P, T], f32, tag="score", name=f"zq{i}_{layer}")
                  for i in range(2)]
            for half in range(2):
                for mm in range(4):
                    m = 4 * half + mm
                    for kd in range(ND):
                        tzp = psA.tile([P, P], f32r, tag="psA", name="tzp")
                        nc.tensor.transpose(
                            tzp[:], z_sb[kd][:, ts(m, P)], identr[:])
                        nc.scalar.copy(
                            zq[half][:, mm * D + kd * P:mm * D + (kd + 1) * P],
                            tzp[:])
            zin = [dpool.tile([P, T], f32, tag=f"zin{h}", name=f"zin{h}_{layer}")
                   for h in range(2)]
            zout = [dpool.tile([P, T], f32, tag=f"zout{h}",
                               name=f"zout{h}_{layer}") for h in range(2)]
            zr = [None, None]
            for half in range(2):
                nc.sync.dma_start(zin[half][:], zq[half][:])
                nc.gpsimd.collective_compute(
                    "AllReduce", mybir.AluOpType.add,
                    ins=[zin[half].opt()], outs=[zout[half].opt()],
                    replica_groups=rg)
                zrh = zpool.tile([P, T], f32, tag=f"z{half}",
                                 name=f"zr{half}_{layer}")
                nc.sync.dma_start(zrh[:], zout[half][:])
                zr[half] = zrh
            keep_pe_warm(56, f"ar{layer}")
            for m in range(NT):
                zb = zr[m // 4][:, ts(m % 4, D)]
                u = lnpool.tile([P, D], f32, tag="u", name=f"u{m}")
                layer_norm(zb, u[:])
                w = lnpool.tile([P, D], f32, tag="w", name=f"w{m}")
                nc.vector.tensor_add(w[:], v_sb[m][:].bitcast(f32), u[:])
                layer_norm(w[:], v_sb[m][:])
            transpose_v()

        # ---- readout ----
        ro_sb = [stream_rounded(d_ro, k, V, f"ro{k}") for k in range(ND)]
        for m in range(NT):
            rps = psA.tile([P, V], f32, tag="psA", name="rps")
            for k in range(ND):
                nc.tensor.matmul(rps[:], vT_sb[k][:, ts(m, P)], ro_sb[k][:],
                                 start=(k == 0), stop=(k == ND - 1))
            o_sb = lnpool.tile([P, V], f32, tag="o", name=f"o{m}")
            nc.scalar.copy(o_sb[:], rps[:])
            nc.sync.dma_start(d_out[ts(m, P), :], o_sb[:])

    nc.compile()
    return nc


def _get_program():
    if "nc" not in _CACHE:
        _CACHE["nc"] = _build_program()
    return _CACHE["nc"]


def _rope_tables():
    inv = (1.0 / (10000.0 ** (np.arange(0, Dh, 2, dtype=np.float32) / Dh)))
    tt = np.arange(T, dtype=np.float32)
    freqs = np.outer(tt, inv).astype(np.float32)  # [T, Dh/2]
    cosT = np.ascontiguousarray(np.cos(freqs).T, dtype=np.float32)
    sinT = np.ascontiguousarray(np.sin(freqs).T, dtype=np.float32)
    return cosT, sinT


def kernel(**inputs):
    global LAST_RESULT
    from concourse import bass_utils

    tokens = np.asarray(inputs["tokens"])
    emb_w = np.ascontiguousarray(inputs["emb_w"], dtype=np.float32)
    E = np.ascontiguousarray(inputs["E"], dtype=np.float32)
    Dx = np.ascontiguousarray(inputs["Dx"], dtype=np.float32)
    Dy = np.ascontiguousarray(inputs["Dy"], dtype=np.float32)
    readout = np.ascontiguousarray(inputs["readout"], dtype=np.float32)

    cosT, sinT = _rope_tables()

    in_maps = []
    for c in range(NCORES):
        b, hp = c // 2, c % 2
        oh = np.zeros((V, T), dtype=np.float32)
        oh[np.asarray(tokens[b], dtype=np.int64), np.arange(T)] = 1.0
        in_maps.append({
            "onehotT": oh,
            "emb_w": emb_w,
            "dx": np.ascontiguousarray(
                Dx[2 * hp:2 * hp + 2].reshape(2 * D, Dh)),
            "dy": np.ascontiguousarray(
                Dy[2 * hp:2 * hp + 2].reshape(2 * D, Dh)),
            "eh": np.ascontiguousarray(E[2 * hp * Dh:(2 * hp + 2) * Dh]),
            "cosT": cosT,
            "sinT": sinT,
            "readout": readout,
        })

    nc = _get_program()
    res = bass_utils.run_bass_kernel_spmd(
        nc, in_maps, core_ids=list(range(NCORES)),
        trace=bool(int(os.environ.get("KERNEL_TRACE", "0"))))
    LAST_RESULT = res
    out = np.stack([res.results[2 * b]["out"] for b in range(B)], axis=0)
    return out



# revision 17
# speedup vs baseline: 1.1251x; 1.1251x over previous
# Trainium2 Bass kernel for nn_BDH_66056597013022 (dense_transformer).
#
# Model (per reference):
#   v = LN(emb_w[tokens])                                  [B,T,D]
#   6x: x  = relu(v @ Dx_h)            per head            [B,H,T,Dh]
#       xr = RoPE(x)
#       S  = xr @ xr^T                 (no softmax)        [B,H,T,T]
#       a  = S @ v                                         [B,H,T,D]
#       y  = relu(a @ Dy_h) * x                            [B,H,T,Dh]
#       v  = LN(v + LN(concat_h(y) @ E))
#   out = v @ readout                                      [B,T,V]
#
# Shapes: B=4 T=1024 H=4 N=4096 D=256 L=6 V=256, Dh=N/H=1024.
#
# Sharding (8 cores): core c -> batch b=c//2, head-pair hp=c%2 (heads 2hp,2hp+1).
# The only cross-core coupling is the head-sum in z = y @ E, reduced with a
# 2-rank AllReduce between cores {2b,2b+1}; both cores then redundantly do the
# LN/v update. Even cores' outputs are returned.
#
# Precision: weights and most activations (x, xr, a, y, z, vT) are bf16
# (PE streams bf16 at 1 cyc/row like f32r; DVE gets the 2x packed mode for
# RoPE; SBUF/DMA halve). v and S stay f32r (the PE forbids mixing 32-bit and
# 16-bit matmul operands, so the a = S @ v contraction pairs f32r with f32r).
# PSUM accumulation is fp32 everywhere. Simulated end-to-end rel err ~1e-2
# vs the 2e-2 gate.
#
# Schedule (the point of this rewrite): attention is computed per
# token-column half (512 cols), z accumulates in PSUM across both local
# heads, and each half's AllReduce is launched as soon as that half's z is
# done. AR(half0) hides under half1's S/a/y/z; AR(half1) hides under the
# next layer's x+RoPE on half0, which only need half0's updated v. The LN
# v-update for a half is emitted at a point where its AR has surely landed.
# This removes the baseline's keep_pe_warm filler and its ~30us/layer of PE
# idle at RoPE and the layer boundary.

import os
import numpy as np

B, T, H, N, D, L, V = 4, 1024, 4, 4096, 256, 6, 256
Dh = N // H
EPS = 1e-5
NCORES = 8
P = 128
NT = T // P    # 8 token blocks
ND = D // P    # 2 model-dim blocks
NDh = Dh // P  # 8 head-dim blocks
HW = 512       # half width (token columns)
NHB = 4        # token blocks per half

_CACHE = {}
LAST_RESULT = None


def _build_program():
    from contextlib import ExitStack

    import concourse.bass as bass
    import concourse.bacc as bacc
    import concourse.tile as tile
    import concourse.mybir as mybir
    from concourse.masks import make_identity

    f32 = mybir.dt.float32
    f32r = mybir.dt.float32r
    bf16 = mybir.dt.bfloat16
    AF = mybir.ActivationFunctionType
    ALU = mybir.AluOpType
    ts = bass.ts

    nc = bacc.Bacc("TRN2", target_bir_lowering=False, debug=False,
                   enable_asserts=False, num_devices=NCORES)

    d_oh = nc.dram_tensor("onehotT", [V, T], bf16, kind="ExternalInput").ap()
    d_ew = nc.dram_tensor("emb_w", [V, D], bf16, kind="ExternalInput").ap()
    d_dx = nc.dram_tensor("dx", [2 * D, Dh], bf16, kind="ExternalInput").ap()
    d_dy = nc.dram_tensor("dy", [2 * D, Dh], bf16, kind="ExternalInput").ap()
    d_eh = nc.dram_tensor("eh", [2 * Dh, D], bf16, kind="ExternalInput").ap()
    d_cos = nc.dram_tensor("cosT", [Dh // 2, T], bf16, kind="ExternalInput").ap()
    d_sin = nc.dram_tensor("sinT", [Dh // 2, T], bf16, kind="ExternalInput").ap()
    d_ro = nc.dram_tensor("readout", [D, V], bf16, kind="ExternalInput").ap()
    d_out = nc.dram_tensor("out", [T, V], f32, kind="ExternalOutput").ap()

    rg = [[0, 1], [2, 3], [4, 5], [6, 7]]

    with tile.TileContext(nc) as tc, ExitStack() as ctx:
        wpool = ctx.enter_context(tc.tile_pool(name="weights", bufs=1))
        vpool = ctx.enter_context(tc.tile_pool(name="vpool", bufs=2))
        xpool = ctx.enter_context(tc.tile_pool(name="xpool", bufs=1))
        xrpool = ctx.enter_context(tc.tile_pool(name="xrpool", bufs=1))
        spool = ctx.enter_context(tc.tile_pool(name="spool", bufs=8))
        apool = ctx.enter_context(tc.tile_pool(name="apool", bufs=4))
        ypool = ctx.enter_context(tc.tile_pool(name="ypool", bufs=3))
        zpool = ctx.enter_context(tc.tile_pool(name="zpool", bufs=2))
        lnpool = ctx.enter_context(tc.tile_pool(name="lnpool", bufs=3))
        stpool = ctx.enter_context(tc.tile_pool(name="stpool", bufs=4))
        psS = ctx.enter_context(tc.tile_pool(name="psS", bufs=2, space="PSUM"))
        psA = ctx.enter_context(tc.tile_pool(name="psA", bufs=2, space="PSUM"))
        psY = ctx.enter_context(tc.tile_pool(name="psY", bufs=2, space="PSUM"))
        psZ = ctx.enter_context(tc.tile_pool(name="psZ", bufs=2, space="PSUM"))
        dpool = ctx.enter_context(tc.tile_pool(name="drampool", bufs=2, space="DRAM"))

        # ---- persistent weights (all bf16, DMA straight in) ----
        def load_bf16(dram_ap, n_tiles, width, tag):
            tiles = []
            for i in range(n_tiles):
                wt = wpool.tile([P, width], bf16, tag=f"{tag}{i}", name=f"{tag}{i}")
                nc.sync.dma_start(wt[:], dram_ap[ts(i, P), :])
                tiles.append(wt)
            return tiles

        oh_sb = load_bf16(d_oh, ND, T, "oh")
        ew_sb = load_bf16(d_ew, ND, D, "ew")
        dx_sb = load_bf16(d_dx, 2 * ND, Dh, "dx")
        cos_sb = load_bf16(d_cos, 4, T, "cos")
        sin_sb = load_bf16(d_sin, 4, T, "sin")
        dy_sb = load_bf16(d_dy, 2 * ND, Dh, "dy")
        eh_sb = load_bf16(d_eh, 2 * NDh, D, "eh")
        ro_sb = load_bf16(d_ro, ND, V, "ro")

        ident = wpool.tile([P, P], f32, tag="ident", name="ident")
        make_identity(nc, ident)
        identr = wpool.tile([P, P], f32r, tag="identr", name="identr")
        nc.scalar.copy(identr[:], ident[:])
        identb = wpool.tile([P, P], bf16, tag="identb", name="identb")
        nc.scalar.copy(identb[:], ident[:])
        epsc = wpool.tile([P, 1], f32, tag="epsc", name="epsc")
        nc.gpsimd.memset(epsc[:], EPS)

        # ---- persistent activations ----
        # v is generation-buffered (ring of 2 per token block): the half-0 LN
        # update for layer l+1 is emitted while head 1 of layer l still reads
        # layer l's v, so updates go to a fresh ring slot and v_cur is
        # repointed. Attention snapshots v_cur at layer start.
        v_cur = [vpool.tile([P, D], f32r, tag=f"v{m}", name=f"v{m}")
                 for m in range(NT)]
        vT_sb = [vpool.tile([P, T], bf16, tag=f"vT{k}", name=f"vT{k}")
                 for k in range(ND)]
        xT = [[xpool.tile([P, T], bf16, tag=f"x{j}_{m}", name=f"x{j}_{m}")
               for m in range(NDh)] for j in range(2)]
        xr = [[xrpool.tile([P, T], bf16, tag=f"xr{j}_{m}", name=f"xr{j}_{m}")
               for m in range(NDh)] for j in range(2)]

        def layer_norm(src_ap, dst_ap):
            st6 = stpool.tile([P, 6], f32, tag="st6", name="st6")
            nc.vector.bn_stats(st6[:], src_ap)
            mv = stpool.tile([P, 2], f32, tag="mv", name="mv")
            nc.vector.bn_aggr(mv[:], st6[:])
            sd = stpool.tile([P, 1], f32, tag="sd", name="sd")
            nc.scalar.activation(sd[:], mv[:, 1:2], AF.Sqrt, bias=epsc[:], scale=1.0)
            rstd = stpool.tile([P, 1], f32, tag="rstd", name="rstd")
            nc.vector.reciprocal(rstd[:], sd[:])
            nmr = stpool.tile([P, 1], f32, tag="nmr", name="nmr")
            nc.vector.scalar_tensor_tensor(
                nmr[:], mv[:, 0:1], -1.0, rstd[:], op0=ALU.mult, op1=ALU.mult)
            nc.scalar.activation(dst_ap, src_ap, AF.Identity,
                                 bias=nmr[:], scale=rstd[:])

        def transpose_v(m):
            # vT[d, t-block m] <- v[m][:, d] for both d-tiles (f32r transpose)
            for d in range(ND):
                tps = psS.tile([P, P], f32r, tag="psS", name="tvps")
                nc.tensor.transpose(tps[:], v_cur[m][:, ts(d, P)], identr[:])
                nc.scalar.copy(vT_sb[d][:, ts(m, P)], tps[:].bitcast(f32))

        def emit_x_half(h):
            # x[:, half h] = relu(Dx^T @ vT[:, h]) for both heads, plus RoPE
            # on that half (DVE, bf16 2x mode). Emitted per pair (m, m+4) so
            # RoPE streams behind the PE.
            c0 = h * HW
            for j in range(2):
                for m in range(4):
                    for mm in (m, m + 4):
                        xps = psS.tile([P, HW], f32, tag="psS", name="xps")
                        for k in range(ND):
                            nc.tensor.matmul(
                                xps[:],
                                dx_sb[2 * j + k][:, ts(mm, P)],
                                vT_sb[k][:, c0:c0 + HW],
                                start=(k == 0), stop=(k == ND - 1))
                        nc.scalar.activation(
                            xT[j][mm][:, c0:c0 + HW], xps[:], AF.Relu)
                    lo = xT[j][m][:, c0:c0 + HW]
                    hi = xT[j][m + 4][:, c0:c0 + HW]
                    cm = cos_sb[m][:, c0:c0 + HW]
                    sm = sin_sb[m][:, c0:c0 + HW]
                    xrl = xr[j][m][:, c0:c0 + HW]
                    xrh = xr[j][m + 4][:, c0:c0 + HW]
                    t1 = lnpool.tile([P, HW], bf16, tag="ropet", name="rt1")
                    nc.vector.tensor_mul(t1[:], hi, sm)
                    nc.vector.tensor_mul(xrl, lo, cm)
                    nc.vector.tensor_sub(xrl, xrl, t1[:])
                    t2 = lnpool.tile([P, HW], bf16, tag="ropet", name="rt2")
                    nc.vector.tensor_mul(t2[:], lo, sm)
                    nc.vector.tensor_mul(xrh, hi, cm)
                    nc.vector.tensor_add(xrh, xrh, t2[:])

        def emit_ln_half(h, zr):
            # v[m] = LN(v[m] + LN(z[m])) for the 4 token blocks of half h,
            # into a fresh ring slot, then refresh vT columns.
            for mm in range(NHB):
                m = NHB * h + mm
                zb = zr[:, mm * D:(mm + 1) * D]
                u = lnpool.tile([P, D], f32, tag="u", name=f"u{m}")
                layer_norm(zb, u[:])
                w = lnpool.tile([P, D], f32, tag="w", name=f"w{m}")
                nc.vector.tensor_add(w[:], v_cur[m][:].bitcast(f32), u[:])
                vn = vpool.tile([P, D], f32r, tag=f"v{m}", name=f"v{m}")
                layer_norm(w[:], vn[:])
                v_cur[m] = vn
            for mm in range(NHB):
                transpose_v(NHB * h + mm)

        def emit_attn_half(h, j, z_ps, v_l, layer):
            # S/a/y for head j on token-column half h; z accumulates into
            # z_ps (shared across both heads; start on j==0 k==0). v_l is the
            # layer's v snapshot (v_cur may already point at l+1 tiles).
            c0 = h * HW
            s_tiles = []
            aT_ps = [psA.tile([P, HW], f32, tag="psA", name=f"aTps{m}")
                     for m in range(ND)]
            for k in range(NT):
                sps = psS.tile([P, HW], f32, tag="psS", name="sps")
                for kk in range(NDh):
                    nc.tensor.matmul(
                        sps[:],
                        xr[j][kk][:, ts(k, P)],
                        xr[j][kk][:, c0:c0 + HW],
                        start=(kk == 0), stop=(kk == NDh - 1))
                s_sb = spool.tile([P, HW], f32r, tag="score", name=f"s{k}")
                nc.scalar.copy(s_sb[:], sps[:])
                s_tiles.append(s_sb)
                # aT accumulation trails by one row block
                if k > 0:
                    for m in range(ND):
                        nc.tensor.matmul(
                            aT_ps[m][:], v_l[k - 1][:, ts(m, P)],
                            s_tiles[k - 1][:],
                            start=(k - 1 == 0), stop=False)
            for m in range(ND):
                nc.tensor.matmul(
                    aT_ps[m][:], v_l[NT - 1][:, ts(m, P)], s_tiles[NT - 1][:],
                    start=False, stop=True)
            aT = []
            for m in range(ND):
                at = apool.tile([P, HW], bf16, tag="aT", name=f"aT{m}")
                nc.scalar.copy(at[:], aT_ps[m][:])
                aT.append(at)
            y_tiles = []
            for k in range(NDh):
                yps = psY.tile([P, HW], f32, tag="psY", name="yps")
                for kk in range(ND):
                    nc.tensor.matmul(
                        yps[:],
                        dy_sb[2 * j + kk][:, ts(k, P)],
                        aT[kk][:],
                        start=(kk == 0), stop=(kk == ND - 1))
                y_sb = ypool.tile([P, HW], bf16, tag="yT", name=f"y{k}")
                nc.vector.scalar_tensor_tensor(
                    y_sb[:], yps[:], 0.0, xT[j][k][:, c0:c0 + HW],
                    op0=ALU.max, op1=ALU.mult)
                y_tiles.append(y_sb)
                if k > 0:
                    for m in range(ND):
                        nc.tensor.matmul(
                            z_ps[m][:], eh_sb[NDh * j + k - 1][:, ts(m, P)],
                            y_tiles[k - 1][:],
                            start=(j == 0 and k - 1 == 0), stop=False)
            for m in range(ND):
                nc.tensor.matmul(
                    z_ps[m][:], eh_sb[NDh * j + NDh - 1][:, ts(m, P)],
                    y_tiles[NDh - 1][:],
                    start=False, stop=(j == 1))

        def emit_z_allreduce(h, z_ps, layer):
            # Evict z (bf16), transpose to [T-half, D] packed layout, DMA to
            # DRAM, AllReduce over the core pair, DMA the sum back.
            z_sb = [zpool.tile([P, HW], bf16, tag=f"zsb{m}",
                               name=f"zsb{m}_{layer}_{h}") for m in range(ND)]
            for m in range(ND):
                nc.scalar.copy(z_sb[m][:], z_ps[m][:])
            zq = zpool.tile([P, T], bf16, tag="zq", name=f"zq{layer}_{h}")
            for mm in range(NHB):
                for kd in range(ND):
                    tzp = psY.tile([P, P], bf16, tag="psY", name="tzp")
                    nc.tensor.transpose(
                        tzp[:], z_sb[kd][:, ts(mm, P)], identb[:])
                    nc.scalar.copy(
                        zq[:, mm * D + kd * P:mm * D + (kd + 1) * P], tzp[:])
            zin = dpool.tile([P, T], bf16, tag=f"zin{h}", name=f"zin{h}_{layer}")
            zout = dpool.tile([P, T], bf16, tag=f"zout{h}",
                              name=f"zout{h}_{layer}")
            nc.sync.dma_start(zin[:], zq[:])
            nc.gpsimd.collective_compute(
                "AllReduce", mybir.AluOpType.add,
                ins=[zin.opt()], outs=[zout.opt()],
                replica_groups=rg)
            zr = zpool.tile([P, T], bf16, tag=f"zr{h}", name=f"zr{h}_{layer}")
            nc.sync.dma_start(zr[:], zout[:])
            return zr

        # ---- embedding: v = LN(onehot @ emb_w) ----
        for m in range(NT):
            eps_t = psS.tile([P, D], f32, tag="psS", name="embps")
            for k in range(ND):
                nc.tensor.matmul(eps_t[:], oh_sb[k][:, ts(m, P)], ew_sb[k][:],
                                 start=(k == 0), stop=(k == ND - 1))
            emb_t = lnpool.tile([P, D], f32, tag="w", name="embt")
            nc.scalar.copy(emb_t[:], eps_t[:])
            layer_norm(emb_t[:], v_cur[m][:])
            transpose_v(m)

        # ---- layers, software-pipelined across the per-half AllReduces ----
        # Invariant entering layer l: x/RoPE for half 0 of layer l are
        # emitted; for l>0 the AR for half 1 of layer l-1 is in flight and
        # its LN update has NOT been emitted yet.
        emit_x_half(0)
        zr_pend = None  # half-1 AR result of the previous layer
        for layer in range(L):
            if zr_pend is not None:
                emit_ln_half(1, zr_pend)  # waits on the in-flight AR
            v_l = list(v_cur)  # this layer's v generation
            emit_x_half(1)
            z_ps = [psZ.tile([P, HW], f32, tag="psZ", name=f"zps{m}")
                    for m in range(ND)]
            for j in range(2):
                emit_attn_half(0, j, z_ps, v_l, layer)
            zr0 = emit_z_allreduce(0, z_ps, layer)
            z_ps = [psZ.tile([P, HW], f32, tag="psZ", name=f"zps{m}b")
                    for m in range(ND)]
            emit_attn_half(1, 0, z_ps, v_l, layer)
            emit_ln_half(0, zr0)  # AR(half0) has landed behind half1's work
            emit_attn_half(1, 1, z_ps, v_l, layer)
            zr_pend = emit_z_allreduce(1, z_ps, layer)
            if layer < L - 1:
                emit_x_half(0)

        # ---- readout (half 0 first; half 1 after the final LN lands) ----
        def emit_readout_half(h):
            for mm in range(NHB):
                m = NHB * h + mm
                rps = psS.tile([P, V], f32, tag="psS", name="rps")
                for k in range(ND):
                    nc.tensor.matmul(rps[:], vT_sb[k][:, ts(m, P)], ro_sb[k][:],
                                     start=(k == 0), stop=(k == ND - 1))
                o_sb = lnpool.tile([P, V], f32, tag="o", name=f"o{m}")
                nc.scalar.copy(o_sb[:], rps[:])
                nc.sync.dma_start(d_out[ts(m, P), :], o_sb[:])

        emit_readout_half(0)
        emit_ln_half(1, zr_pend)
        emit_readout_half(1)

    nc.compile()
    return nc


def _get_program():
    if "nc" not in _CACHE:
        _CACHE["nc"] = _build_program()
    return _CACHE["nc"]


def _rope_tables():
    inv = (1.0 / (10000.0 ** (np.arange(0, Dh, 2, dtype=np.float32) / Dh)))
    tt = np.arange(T, dtype=np.float32)
    freqs = np.outer(tt, inv).astype(np.float32)  # [T, Dh/2]
    cosT = np.ascontiguousarray(np.cos(freqs).T, dtype=np.float32)
    sinT = np.ascontiguousarray(np.sin(freqs).T, dtype=np.float32)
    return cosT, sinT


def kernel(**inputs):
    global LAST_RESULT
    import ml_dtypes
    from concourse import bass_utils

    bf = ml_dtypes.bfloat16
    tokens = np.asarray(inputs["tokens"])
    emb_w = np.ascontiguousarray(inputs["emb_w"], dtype=np.float32)
    E = np.ascontiguousarray(inputs["E"], dtype=np.float32)
    Dx = np.ascontiguousarray(inputs["Dx"], dtype=np.float32)
    Dy = np.ascontiguousarray(inputs["Dy"], dtype=np.float32)
    readout = np.ascontiguousarray(inputs["readout"], dtype=np.float32)

    cosT, sinT = _rope_tables()

    in_maps = []
    for c in range(NCORES):
        b, hp = c // 2, c % 2
        oh = np.zeros((V, T), dtype=bf)
        oh[np.asarray(tokens[b], dtype=np.int64), np.arange(T)] = 1.0
        in_maps.append({
            "onehotT": oh,
            "emb_w": emb_w.astype(bf),
            "dx": np.ascontiguousarray(
                Dx[2 * hp:2 * hp + 2].reshape(2 * D, Dh).astype(bf)),
            "dy": np.ascontiguousarray(
                Dy[2 * hp:2 * hp + 2].reshape(2 * D, Dh).astype(bf)),
            "eh": np.ascontiguousarray(
                E[2 * hp * Dh:(2 * hp + 2) * Dh].astype(bf)),
            "cosT": cosT.astype(bf),
            "sinT": sinT.astype(bf),
            "readout": readout.astype(bf),
        })

    nc = _get_program()
    res = bass_utils.run_bass_kernel_spmd(
        nc, in_maps, core_ids=list(range(NCORES)),
        trace=bool(int(os.environ.get("KERNEL_TRACE", "0"))))
    LAST_RESULT = res
    out = np.stack([res.results[2 * b]["out"] for b in range(B)], axis=0)
    return out


# revision 26
# speedup vs baseline: 1.2201x; 1.0845x over previous
# Trainium2 Bass kernel for nn_BDH_66056597013022 (dense_transformer).
#
# Model (per reference):
#   v = LN(emb_w[tokens])                                  [B,T,D]
#   6x: x  = relu(v @ Dx_h)            per head            [B,H,T,Dh]
#       xr = RoPE(x)
#       S  = xr @ xr^T                 (no softmax)        [B,H,T,T]
#       a  = S @ v                                         [B,H,T,D]
#       y  = relu(a @ Dy_h) * x                            [B,H,T,Dh]
#       v  = LN(v + LN(concat_h(y) @ E))
#   out = v @ readout                                      [B,T,V]
#
# Shapes: B=4 T=1024 H=4 N=4096 D=256 L=6 V=256, Dh=N/H=1024.
#
# Sharding (8 cores): core c -> batch b=c//2, head-pair hp=c%2 (heads 2hp,2hp+1).
# The only cross-core coupling is the head-sum in z = y @ E, reduced with a
# 2-rank AllReduce between cores {2b,2b+1}; both cores then redundantly do the
# LN/v update. Even cores' outputs are returned.
#
# Precision: weights and most activations (x, xr, a, y, z, vT) are bf16
# (PE streams bf16 at 1 cyc/row like f32r; DVE gets the 2x packed mode for
# RoPE; SBUF/DMA halve). v and S stay f32r (the PE forbids mixing 32-bit and
# 16-bit matmul operands, so the a = S @ v contraction pairs f32r with f32r).
# PSUM accumulation is fp32 everywhere. Simulated end-to-end rel err ~1e-2
# vs the 2e-2 gate.
#
# Schedule (the point of this rewrite): attention is computed per
# token-column half (512 cols), z accumulates in PSUM across both local
# heads, and each half's AllReduce is launched as soon as that half's z is
# done. AR(half0) hides under half1's S/a/y/z; AR(half1) hides under the
# next layer's x+RoPE on half0, which only need half0's updated v. The LN
# v-update for a half is emitted at a point where its AR has surely landed.
# This removes the baseline's keep_pe_warm filler and its ~30us/layer of PE
# idle at RoPE and the layer boundary.

import os
import numpy as np

B, T, H, N, D, L, V = 4, 1024, 4, 4096, 256, 6, 256
Dh = N // H
EPS = 1e-5
NCORES = 8
P = 128
NT = T // P    # 8 token blocks
ND = D // P    # 2 model-dim blocks
NDh = Dh // P  # 8 head-dim blocks
HW = 512       # half width (token columns)
NHB = 4        # token blocks per half

_CACHE = {}
LAST_RESULT = None


def _build_program():
    from contextlib import ExitStack

    import concourse.bass as bass
    import concourse.bacc as bacc
    import concourse.tile as tile
    import concourse.mybir as mybir
    from concourse.masks import make_identity

    f32 = mybir.dt.float32
    f32r = mybir.dt.float32r
    bf16 = mybir.dt.bfloat16
    AF = mybir.ActivationFunctionType
    ALU = mybir.AluOpType
    ts = bass.ts

    nc = bacc.Bacc("TRN2", target_bir_lowering=False, debug=False,
                   enable_asserts=False, num_devices=NCORES)

    d_oh = nc.dram_tensor("onehotT", [V, T], bf16, kind="ExternalInput").ap()
    d_ew = nc.dram_tensor("emb_w", [V, D], bf16, kind="ExternalInput").ap()
    d_dx = nc.dram_tensor("dx", [2 * D, Dh], bf16, kind="ExternalInput").ap()
    d_dy = nc.dram_tensor("dy", [2 * D, Dh], bf16, kind="ExternalInput").ap()
    d_eh = nc.dram_tensor("eh", [2 * Dh, D], bf16, kind="ExternalInput").ap()
    d_cos = nc.dram_tensor("cosT", [Dh // 2, T], bf16, kind="ExternalInput").ap()
    d_sin = nc.dram_tensor("sinT", [Dh // 2, T], bf16, kind="ExternalInput").ap()
    d_ro = nc.dram_tensor("readout", [D, V], bf16, kind="ExternalInput").ap()
    d_out = nc.dram_tensor("out", [T, V], f32, kind="ExternalOutput").ap()

    rg = [[0, 1], [2, 3], [4, 5], [6, 7]]

    with tile.TileContext(nc) as tc, ExitStack() as ctx:
        wpool = ctx.enter_context(tc.tile_pool(name="weights", bufs=1))
        vpool = ctx.enter_context(tc.tile_pool(name="vpool", bufs=2))
        xpool = ctx.enter_context(tc.tile_pool(name="xpool", bufs=1))
        xrpool = ctx.enter_context(tc.tile_pool(name="xrpool", bufs=1))
        spool = ctx.enter_context(tc.tile_pool(name="spool", bufs=12))
        apool = ctx.enter_context(tc.tile_pool(name="apool", bufs=4))
        ypool = ctx.enter_context(tc.tile_pool(name="ypool", bufs=3))
        zpool = ctx.enter_context(tc.tile_pool(name="zpool", bufs=2))
        lnpool = ctx.enter_context(tc.tile_pool(name="lnpool", bufs=3))
        stpool = ctx.enter_context(tc.tile_pool(name="stpool", bufs=4))
        psS = ctx.enter_context(tc.tile_pool(name="psS", bufs=2, space="PSUM"))
        psA = ctx.enter_context(tc.tile_pool(name="psA", bufs=2, space="PSUM"))
        psY = ctx.enter_context(tc.tile_pool(name="psY", bufs=2, space="PSUM"))
        psZ = ctx.enter_context(tc.tile_pool(name="psZ", bufs=2, space="PSUM"))
        dpool = ctx.enter_context(tc.tile_pool(name="drampool", bufs=2, space="DRAM"))

        # ---- persistent weights (all bf16, DMA straight in) ----
        def load_bf16(dram_ap, n_tiles, width, tag):
            tiles = []
            for i in range(n_tiles):
                wt = wpool.tile([P, width], bf16, tag=f"{tag}{i}", name=f"{tag}{i}")
                nc.sync.dma_start(wt[:], dram_ap[ts(i, P), :])
                tiles.append(wt)
            return tiles

        oh_sb = load_bf16(d_oh, ND, T, "oh")
        ew_sb = load_bf16(d_ew, ND, D, "ew")
        dx_sb = load_bf16(d_dx, 2 * ND, Dh, "dx")
        cos_sb = load_bf16(d_cos, 4, T, "cos")
        sin_sb = load_bf16(d_sin, 4, T, "sin")
        dy_sb = load_bf16(d_dy, 2 * ND, Dh, "dy")
        eh_sb = load_bf16(d_eh, 2 * NDh, D, "eh")
        ro_sb = load_bf16(d_ro, ND, V, "ro")

        ident = wpool.tile([P, P], f32, tag="ident", name="ident")
        make_identity(nc, ident)
        identr = wpool.tile([P, P], f32r, tag="identr", name="identr")
        nc.scalar.copy(identr[:], ident[:])
        identb = wpool.tile([P, P], bf16, tag="identb", name="identb")
        nc.scalar.copy(identb[:], ident[:])
        epsc = wpool.tile([P, 1], f32, tag="epsc", name="epsc")
        nc.gpsimd.memset(epsc[:], EPS)

        # Fire a tiny AllReduce during the weight DMAs so the first real
        # collective doesn't pay the ncfw warm-up latency.
        warm_sb = zpool.tile([P, 16], f32, tag="warmsb", name="warmsb")
        nc.gpsimd.memset(warm_sb[:], 0.0)
        warm_in = dpool.tile([P, 16], f32, tag="warmin", name="warmin")
        warm_out = dpool.tile([P, 16], f32, tag="warmout", name="warmout")
        nc.sync.dma_start(warm_in[:], warm_sb[:])
        nc.gpsimd.collective_compute(
            "AllReduce", mybir.AluOpType.add,
            ins=[warm_in.opt()], outs=[warm_out.opt()], replica_groups=rg)

        # ---- persistent activations ----
        # v is generation-buffered (ring of 2 per token block): the half-0 LN
        # update for layer l+1 is emitted while head 1 of layer l still reads
        # layer l's v, so updates go to a fresh ring slot and v_cur is
        # repointed. Attention snapshots v_cur at layer start.
        v_cur = [vpool.tile([P, D], f32r, tag=f"v{m}", name=f"v{m}")
                 for m in range(NT)]
        vT_sb = [vpool.tile([P, T], bf16, tag=f"vT{k}", name=f"vT{k}")
                 for k in range(ND)]
        xT = [[xpool.tile([P, T], bf16, tag=f"x{j}_{m}", name=f"x{j}_{m}")
               for m in range(NDh)] for j in range(2)]
        xr = [[xrpool.tile([P, T], bf16, tag=f"xr{j}_{m}", name=f"xr{j}_{m}")
               for m in range(NDh)] for j in range(2)]

        def rstd_of(src_ap):
            # 1/sqrt(var(src) + eps) as a [P,1] per-partition vector
            st6 = stpool.tile([P, 6], f32, tag="st6", name="st6")
            nc.vector.bn_stats(st6[:], src_ap)
            mv = stpool.tile([P, 2], f32, tag="mv", name="mv")
            nc.vector.bn_aggr(mv[:], st6[:])
            sd = stpool.tile([P, 1], f32, tag="sd", name="sd")
            nc.scalar.activation(sd[:], mv[:, 1:2], AF.Sqrt,
                                 bias=epsc[:], scale=1.0)
            rstd = stpool.tile([P, 1], f32, tag="rstd", name="rstd")
            nc.vector.reciprocal(rstd[:], sd[:])
            return mv, rstd

        def layer_norm(src_ap, dst_ap):
            mv, rstd = rstd_of(src_ap)
            nmr = stpool.tile([P, 1], f32, tag="nmr", name="nmr")
            nc.vector.scalar_tensor_tensor(
                nmr[:], mv[:, 0:1], -1.0, rstd[:], op0=ALU.mult, op1=ALU.mult)
            nc.scalar.activation(dst_ap, src_ap, AF.Identity,
                                 bias=nmr[:], scale=rstd[:])

        def transpose_v(m):
            # vT[d, t-block m] <- v[m][:, d] for both d-tiles (f32r transpose)
            for d in range(ND):
                tps = psS.tile([P, P], f32r, tag="psS", name="tvps")
                nc.tensor.transpose(tps[:], v_cur[m][:, ts(d, P)], identr[:])
                nc.scalar.copy(vT_sb[d][:, ts(m, P)], tps[:].bitcast(f32))

        def emit_x_half(h):
            # x[:, half h] = relu(Dx^T @ vT[:, h]) for both heads, plus RoPE
            # on that half (bf16 2x mode). Emitted per pair (m, m+4) so RoPE
            # streams behind the PE. Pair 3 runs on the otherwise-idle GPSIMD
            # engine so the per-head RoPE latency is ~3 DVE pairs, not 4.
            c0 = h * HW
            for j in range(2):
                for m in range(4):
                    for mm in (m, m + 4):
                        xps = psS.tile([P, HW], f32, tag="psS", name="xps")
                        for k in range(ND):
                            nc.tensor.matmul(
                                xps[:],
                                dx_sb[2 * j + k][:, ts(mm, P)],
                                vT_sb[k][:, c0:c0 + HW],
                                start=(k == 0), stop=(k == ND - 1))
                        nc.scalar.activation(
                            xT[j][mm][:, c0:c0 + HW], xps[:], AF.Relu)
                    eng = nc.gpsimd if m == 3 else nc.vector
                    lo = xT[j][m][:, c0:c0 + HW]
                    hi = xT[j][m + 4][:, c0:c0 + HW]
                    cm = cos_sb[m][:, c0:c0 + HW]
                    sm = sin_sb[m][:, c0:c0 + HW]
                    xrl = xr[j][m][:, c0:c0 + HW]
                    xrh = xr[j][m + 4][:, c0:c0 + HW]
                    tg = "ropetg" if m == 3 else "ropet"
                    t1 = lnpool.tile([P, HW], bf16, tag=tg, name="rt1")
                    eng.tensor_mul(t1[:], hi, sm)
                    eng.tensor_mul(xrl, lo, cm)
                    eng.tensor_sub(xrl, xrl, t1[:])
                    t2 = lnpool.tile([P, HW], bf16, tag=tg, name="rt2")
                    eng.tensor_mul(t2[:], lo, sm)
                    eng.tensor_mul(xrh, hi, cm)
                    eng.tensor_add(xrh, xrh, t2[:])

        def emit_ln_half(h, zr):
            # v[m] = LN(v[m] + LN(z[m])) for the 4 token blocks of half h,
            # into a fresh ring slot, then refresh vT columns. The inner LN's
            # mean-subtraction is dropped: the outer LN is invariant to a
            # per-row constant shift, so LN(v + LN(z)) == LN(v + z*rstd_z).
            for mm in range(NHB):
                m = NHB * h + mm
                zb = zr[:, mm * D:(mm + 1) * D]
                _, rstd_z = rstd_of(zb)
                w = lnpool.tile([P, D], f32, tag="w", name=f"w{m}")
                nc.vector.scalar_tensor_tensor(
                    w[:], zb, rstd_z[:], v_cur[m][:].bitcast(f32),
                    op0=ALU.mult, op1=ALU.add)
                vn = vpool.tile([P, D], f32r, tag=f"v{m}", name=f"v{m}")
                layer_norm(w[:], vn[:])
                v_cur[m] = vn
            for mm in range(NHB):
                transpose_v(NHB * h + mm)

        def emit_s_rows(h, j, k_lo, k_hi, s_tiles):
            # S row blocks [k_lo, k_hi) for head j, columns of half h.
            # Row block k only needs xr columns of block k as lhsT, so rows
            # 0..3 can run before this layer's half-1 x/RoPE exist.
            c0 = h * HW
            for k in range(k_lo, k_hi):
                sps = psS.tile([P, HW], f32, tag="psS", name="sps")
                for kk in range(NDh):
                    nc.tensor.matmul(
                        sps[:],
                        xr[j][kk][:, ts(k, P)],
                        xr[j][kk][:, c0:c0 + HW],
                        start=(kk == 0), stop=(kk == NDh - 1))
                s_sb = spool.tile([P, HW], f32r, tag="score", name=f"s{k}")
                nc.scalar.copy(s_sb[:], sps[:])
                s_tiles.append(s_sb)

        def emit_ayz(h, j, z_ps, v_l, s_tiles):
            # aT = v^T @ S (uses S's symmetry), y = relu(Dy^T aT) * x, and
            # z += E_h^T y accumulating across both heads in PSUM.
            c0 = h * HW
            aT_ps = [psA.tile([P, HW], f32, tag="psA", name=f"aTps{m}")
                     for m in range(ND)]
            for k in range(NT):
                for m in range(ND):
                    nc.tensor.matmul(
                        aT_ps[m][:], v_l[k][:, ts(m, P)], s_tiles[k][:],
                        start=(k == 0), stop=(k == NT - 1))
            aT = []
            for m in range(ND):
                at = apool.tile([P, HW], bf16, tag="aT", name=f"aT{m}")
                nc.scalar.copy(at[:], aT_ps[m][:])
                aT.append(at)
            y_tiles = []
            for k in range(NDh):
                yps = psY.tile([P, HW], f32, tag="psY", name="yps")
                for kk in range(ND):
                    nc.tensor.matmul(
                        yps[:],
                        dy_sb[2 * j + kk][:, ts(k, P)],
                        aT[kk][:],
                        start=(kk == 0), stop=(kk == ND - 1))
                y_sb = ypool.tile([P, HW], bf16, tag="yT", name=f"y{k}")
                nc.vector.scalar_tensor_tensor(
                    y_sb[:], yps[:], 0.0, xT[j][k][:, c0:c0 + HW],
                    op0=ALU.max, op1=ALU.mult)
                y_tiles.append(y_sb)
                if k > 0:
                    for m in range(ND):
                        nc.tensor.matmul(
                            z_ps[m][:], eh_sb[NDh * j + k - 1][:, ts(m, P)],
                            y_tiles[k - 1][:],
                            start=(j == 0 and k - 1 == 0), stop=False)
            for m in range(ND):
                nc.tensor.matmul(
                    z_ps[m][:], eh_sb[NDh * j + NDh - 1][:, ts(m, P)],
                    y_tiles[NDh - 1][:],
                    start=False, stop=(j == 1))

        def emit_attn_half(h, j, z_ps, v_l, layer):
            s_tiles = []
            emit_s_rows(h, j, 0, NT, s_tiles)
            emit_ayz(h, j, z_ps, v_l, s_tiles)

        def emit_z_allreduce(h, z_ps, layer):
            # Evict z (bf16), transpose to [T-half, D] packed layout, DMA to
            # DRAM, AllReduce over the core pair, DMA the sum back.
            z_sb = [zpool.tile([P, HW], bf16, tag=f"zsb{m}",
                               name=f"zsb{m}_{layer}_{h}") for m in range(ND)]
            for m in range(ND):
                nc.scalar.copy(z_sb[m][:], z_ps[m][:])
            zq = zpool.tile([P, T], bf16, tag="zq", name=f"zq{layer}_{h}")
            for mm in range(NHB):
                for kd in range(ND):
                    tzp = psY.tile([P, P], bf16, tag="psY", name="tzp")
                    nc.tensor.transpose(
                        tzp[:], z_sb[kd][:, ts(mm, P)], identb[:])
                    nc.scalar.copy(
                        zq[:, mm * D + kd * P:mm * D + (kd + 1) * P], tzp[:])
            zin = dpool.tile([P, T], bf16, tag=f"zin{h}", name=f"zin{h}_{layer}")
            zout = dpool.tile([P, T], bf16, tag=f"zout{h}",
                              name=f"zout{h}_{layer}")
            nc.sync.dma_start(zin[:], zq[:])
            nc.gpsimd.collective_compute(
                "AllReduce", mybir.AluOpType.add,
                ins=[zin.opt()], outs=[zout.opt()],
                replica_groups=rg)
            zr = zpool.tile([P, T], bf16, tag=f"zr{h}", name=f"zr{h}_{layer}")
            nc.sync.dma_start(zr[:], zout[:])
            return zr

        # ---- embedding: v = LN(onehot @ emb_w) ----
        for m in range(NT):
            eps_t = psS.tile([P, D], f32, tag="psS", name="embps")
            for k in range(ND):
                nc.tensor.matmul(eps_t[:], oh_sb[k][:, ts(m, P)], ew_sb[k][:],
                                 start=(k == 0), stop=(k == ND - 1))
            emb_t = lnpool.tile([P, D], f32, tag="w", name="embt")
            nc.scalar.copy(emb_t[:], eps_t[:])
            layer_norm(emb_t[:], v_cur[m][:])
            transpose_v(m)

        # ---- layers, software-pipelined across the per-half AllReduces ----
        # Invariant entering layer l: x/RoPE for half 0 of layer l are
        # emitted; for l>0 the AR for half 1 of layer l-1 is in flight and
        # its LN update has NOT been emitted yet. S(half0) rows 0..3 (which
        # need only half-0 RoPE) are emitted before that LN so the PE has
        # AR-independent work while the collective + LN chain drain.
        emit_x_half(0)
        zr_pend = None  # half-1 AR result of the previous layer
        for layer in range(L):
            s00, s01 = [], []
            emit_s_rows(0, 0, 0, NHB, s00)
            emit_s_rows(0, 1, 0, NHB, s01)
            if zr_pend is not None:
                emit_ln_half(1, zr_pend)  # waits on the in-flight AR
            v_l = list(v_cur)  # layer l's v is complete only after the LN
            emit_x_half(1)
            emit_s_rows(0, 0, NHB, NT, s00)
            z_ps = [psZ.tile([P, HW], f32, tag="psZ", name=f"zps{m}")
                    for m in range(ND)]
            emit_ayz(0, 0, z_ps, v_l, s00)
            emit_s_rows(0, 1, NHB, NT, s01)
            emit_ayz(0, 1, z_ps, v_l, s01)
            zr0 = emit_z_allreduce(0, z_ps, layer)
            z_ps = [psZ.tile([P, HW], f32, tag="psZ", name=f"zps{m}b")
                    for m in range(ND)]
            emit_attn_half(1, 0, z_ps, v_l, layer)
            emit_ln_half(0, zr0)  # AR(half0) has landed behind half1's work
            emit_attn_half(1, 1, z_ps, v_l, layer)
            zr_pend = emit_z_allreduce(1, z_ps, layer)
            if layer < L - 1:
                emit_x_half(0)

        # ---- readout (half 0 first; half 1 after the final LN lands) ----
        def emit_readout_half(h):
            for mm in range(NHB):
                m = NHB * h + mm
                rps = psS.tile([P, V], f32, tag="psS", name="rps")
                for k in range(ND):
                    nc.tensor.matmul(rps[:], vT_sb[k][:, ts(m, P)], ro_sb[k][:],
                                     start=(k == 0), stop=(k == ND - 1))
                o_sb = lnpool.tile([P, V], f32, tag="o", name=f"o{m}")
                nc.scalar.copy(o_sb[:], rps[:])
                nc.sync.dma_start(d_out[ts(m, P), :], o_sb[:])

        emit_readout_half(0)
        emit_ln_half(1, zr_pend)
        emit_readout_half(1)

    nc.compile()
    return nc


def _get_program():
    if "nc" not in _CACHE:
        _CACHE["nc"] = _build_program()
    return _CACHE["nc"]


def _rope_tables():
    inv = (1.0 / (10000.0 ** (np.arange(0, Dh, 2, dtype=np.float32) / Dh)))
    tt = np.arange(T, dtype=np.float32)
    freqs = np.outer(tt, inv).astype(np.float32)  # [T, Dh/2]
    cosT = np.ascontiguousarray(np.cos(freqs).T, dtype=np.float32)
    sinT = np.ascontiguousarray(np.sin(freqs).T, dtype=np.float32)
    return cosT, sinT


def kernel(**inputs):
    global LAST_RESULT
    import ml_dtypes
    from concourse import bass_utils

    bf = ml_dtypes.bfloat16
    tokens = np.asarray(inputs["tokens"])
    emb_w = np.ascontiguousarray(inputs["emb_w"], dtype=np.float32)
    E = np.ascontiguousarray(inputs["E"], dtype=np.float32)
    Dx = np.ascontiguousarray(inputs["Dx"], dtype=np.float32)
    Dy = np.ascontiguousarray(inputs["Dy"], dtype=np.float32)
    readout = np.ascontiguousarray(inputs["readout"], dtype=np.float32)

    cosT, sinT = _rope_tables()

    in_maps = []
    for c in range(NCORES):
        b, hp = c // 2, c % 2
        oh = np.zeros((V, T), dtype=bf)
        oh[np.asarray(tokens[b], dtype=np.int64), np.arange(T)] = 1.0
        in_maps.append({
            "onehotT": oh,
            "emb_w": emb_w.astype(bf),
            "dx": np.ascontiguousarray(
                Dx[2 * hp:2 * hp + 2].reshape(2 * D, Dh).astype(bf)),
            "dy": np.ascontiguousarray(
                Dy[2 * hp:2 * hp + 2].reshape(2 * D, Dh).astype(bf)),
            "eh": np.ascontiguousarray(
                E[2 * hp * Dh:(2 * hp + 2) * Dh].astype(bf)),
            "cosT": cosT.astype(bf),
            "sinT": sinT.astype(bf),
            "readout": readout.astype(bf),
        })

    nc = _get_program()
    res = bass_utils.run_bass_kernel_spmd(
        nc, in_maps, core_ids=list(range(NCORES)),
        trace=bool(int(os.environ.get("KERNEL_TRACE", "0"))))
    LAST_RESULT = res
    out = np.stack([res.results[2 * b]["out"] for b in range(B)], axis=0)
    return out


# revision 36
# speedup vs baseline: 1.4679x; 1.2031x over previous
# Trainium2 Bass kernel for nn_BDH_66056597013022 (dense_transformer).
#
# Model (per reference):
#   v = LN(emb_w[tokens])                                  [B,T,D]
#   6x: x  = relu(v @ Dx_h)            per head            [B,H,T,Dh]
#       xr = RoPE(x)
#       S  = xr @ xr^T                 (no softmax)        [B,H,T,T]
#       a  = S @ v                                         [B,H,T,D]
#       y  = relu(a @ Dy_h) * x                            [B,H,T,Dh]
#       v  = LN(v + LN(concat_h(y) @ E))
#   out = v @ readout                                      [B,T,V]
#
# Shapes: B=4 T=1024 H=4 N=4096 D=256 L=6 V=256, Dh=N/H=1024.
#
# Sharding (8 cores): core c -> batch b=c//2, head-pair hp=c%2 (heads 2hp,2hp+1).
# The only cross-core coupling is the head-sum in z = y @ E, reduced with a
# 2-rank AllReduce between cores {2b,2b+1}; both cores then redundantly do the
# LN/v update. Even cores' outputs are returned.
#
# Precision: weights and most activations (x, xr, a, y, z, vT) are bf16
# (PE streams bf16 at 1 cyc/row like f32r; DVE gets the 2x packed mode for
# RoPE; SBUF/DMA halve). v and S stay f32r (the PE forbids mixing 32-bit and
# 16-bit matmul operands, so the a = S @ v contraction pairs f32r with f32r).
# PSUM accumulation is fp32 everywhere. Simulated end-to-end rel err ~1e-2
# vs the 2e-2 gate.
#
# Schedule (the point of this rewrite): attention is computed per
# token-column half (512 cols), z accumulates in PSUM across both local
# heads, and each half's AllReduce is launched as soon as that half's z is
# done. AR(half0) hides under half1's S/a/y/z; AR(half1) hides under the
# next layer's x+RoPE on half0, which only need half0's updated v. The LN
# v-update for a half is emitted at a point where its AR has surely landed.
# This removes the baseline's keep_pe_warm filler and its ~30us/layer of PE
# idle at RoPE and the layer boundary.

import os
import numpy as np

B, T, H, N, D, L, V = 4, 1024, 4, 4096, 256, 6, 256
Dh = N // H
EPS = 1e-5
NCORES = 8
P = 128
NT = T // P    # 8 token blocks
ND = D // P    # 2 model-dim blocks
NDh = Dh // P  # 8 head-dim blocks
HW = 512       # half width (token columns)
NHB = 4        # token blocks per half

_CACHE = {}
LAST_RESULT = None


def _build_program():
    from contextlib import ExitStack

    import concourse.bass as bass
    import concourse.bacc as bacc
    import concourse.tile as tile
    import concourse.mybir as mybir
    from concourse.masks import make_identity

    f32 = mybir.dt.float32
    f32r = mybir.dt.float32r
    bf16 = mybir.dt.bfloat16
    AF = mybir.ActivationFunctionType
    ALU = mybir.AluOpType
    ts = bass.ts

    nc = bacc.Bacc("TRN2", target_bir_lowering=False, debug=False,
                   enable_asserts=False, num_devices=NCORES)

    d_oh = nc.dram_tensor("onehotT", [V, T], bf16, kind="ExternalInput").ap()
    d_ew = nc.dram_tensor("emb_w", [V, D], bf16, kind="ExternalInput").ap()
    d_dx = nc.dram_tensor("dx", [2 * D, Dh], bf16, kind="ExternalInput").ap()
    d_dy = nc.dram_tensor("dy", [2 * D, Dh], bf16, kind="ExternalInput").ap()
    d_eh = nc.dram_tensor("eh", [2 * Dh, D], bf16, kind="ExternalInput").ap()
    d_cos = nc.dram_tensor("cosT", [Dh // 2, T], bf16, kind="ExternalInput").ap()
    d_sin = nc.dram_tensor("sinT", [Dh // 2, T], bf16, kind="ExternalInput").ap()
    d_ro = nc.dram_tensor("readout", [D, V], bf16, kind="ExternalInput").ap()
    d_out = nc.dram_tensor("out", [T, V], f32, kind="ExternalOutput").ap()

    rg = [[0, 1], [2, 3], [4, 5], [6, 7]]

    with tile.TileContext(nc) as tc, ExitStack() as ctx:
        wpool = ctx.enter_context(tc.tile_pool(name="weights", bufs=1))
        vpool = ctx.enter_context(tc.tile_pool(name="vpool", bufs=2))
        xpool = ctx.enter_context(tc.tile_pool(name="xpool", bufs=1))
        xrpool = ctx.enter_context(tc.tile_pool(name="xrpool", bufs=1))
        spool = ctx.enter_context(tc.tile_pool(name="spool", bufs=12))
        apool = ctx.enter_context(tc.tile_pool(name="apool", bufs=4))
        ypool = ctx.enter_context(tc.tile_pool(name="ypool", bufs=3))
        zpool = ctx.enter_context(tc.tile_pool(name="zpool", bufs=2))
        lnpool = ctx.enter_context(tc.tile_pool(name="lnpool", bufs=3))
        stpool = ctx.enter_context(tc.tile_pool(name="stpool", bufs=4))
        psS = ctx.enter_context(tc.tile_pool(name="psS", bufs=2, space="PSUM"))
        psA = ctx.enter_context(tc.tile_pool(name="psA", bufs=2, space="PSUM"))
        psY = ctx.enter_context(tc.tile_pool(name="psY", bufs=2, space="PSUM"))
        psZ = ctx.enter_context(tc.tile_pool(name="psZ", bufs=2, space="PSUM"))
        dpool = ctx.enter_context(tc.tile_pool(name="drampool", bufs=2, space="DRAM"))

        # ---- persistent weights (all bf16, DMA straight in) ----
        def load_bf16(dram_ap, n_tiles, width, tag):
            tiles = []
            for i in range(n_tiles):
                wt = wpool.tile([P, width], bf16, tag=f"{tag}{i}", name=f"{tag}{i}")
                nc.sync.dma_start(wt[:], dram_ap[ts(i, P), :])
                tiles.append(wt)
            return tiles

        oh_sb = load_bf16(d_oh, ND, T, "oh")
        ew_sb = load_bf16(d_ew, ND, D, "ew")
        dx_sb = load_bf16(d_dx, 2 * ND, Dh, "dx")
        # cos/sin packed as [P, 4, T] so RoPE can run as wide multi-block ops
        cos_mg = wpool.tile([P, 4, T], bf16, tag="cosmg", name="cosmg")
        sin_mg = wpool.tile([P, 4, T], bf16, tag="sinmg", name="sinmg")
        for i in range(4):
            nc.sync.dma_start(cos_mg[:, i, :], d_cos[ts(i, P), :])
        for i in range(4):
            nc.sync.dma_start(sin_mg[:, i, :], d_sin[ts(i, P), :])
        dy_sb = load_bf16(d_dy, 2 * ND, Dh, "dy")
        eh_sb = load_bf16(d_eh, 2 * NDh, D, "eh")
        ro_sb = load_bf16(d_ro, ND, V, "ro")

        ident = wpool.tile([P, P], f32, tag="ident", name="ident")
        make_identity(nc, ident)
        identr = wpool.tile([P, P], f32r, tag="identr", name="identr")
        nc.scalar.copy(identr[:], ident[:])
        identb = wpool.tile([P, P], bf16, tag="identb", name="identb")
        nc.scalar.copy(identb[:], ident[:])
        epsc = wpool.tile([P, 1], f32, tag="epsc", name="epsc")
        nc.gpsimd.memset(epsc[:], EPS)

        # Fire a tiny AllReduce during the weight DMAs so the first real
        # collective doesn't pay the ncfw warm-up latency.
        warm_sb = zpool.tile([P, 16], f32, tag="warmsb", name="warmsb")
        nc.gpsimd.memset(warm_sb[:], 0.0)
        warm_in = dpool.tile([P, 16], f32, tag="warmin", name="warmin")
        warm_out = dpool.tile([P, 16], f32, tag="warmout", name="warmout")
        nc.sync.dma_start(warm_in[:], warm_sb[:])
        nc.gpsimd.collective_compute(
            "AllReduce", mybir.AluOpType.add,
            ins=[warm_in.opt()], outs=[warm_out.opt()], replica_groups=rg)

        # ---- persistent activations ----
        # v is generation-buffered (ring of 2 per token block): the half-0 LN
        # update for layer l+1 is emitted while head 1 of layer l still reads
        # layer l's v, so updates go to a fresh ring slot and v_cur is
        # repointed. Attention snapshots v_cur at layer start.
        v_cur = [vpool.tile([P, D], f32r, tag=f"v{m}", name=f"v{m}")
                 for m in range(NT)]
        vT_sb = [vpool.tile([P, T], bf16, tag=f"vT{k}", name=f"vT{k}")
                 for k in range(ND)]
        # x and xr packed [P, NDh, T] per head so RoPE runs as 6 wide ops per
        # (head, half) — the ~230-cycle DVE per-op overhead amortizes 4x.
        xT = [xpool.tile([P, NDh, T], bf16, tag=f"x{j}", name=f"x{j}")
              for j in range(2)]
        xr = [xrpool.tile([P, NDh, T], bf16, tag=f"xr{j}", name=f"xr{j}")
              for j in range(2)]

        def rstd_of(src_ap):
            # 1/sqrt(var(src) + eps) as a [P,1] per-partition vector
            st6 = stpool.tile([P, 6], f32, tag="st6", name="st6")
            nc.vector.bn_stats(st6[:], src_ap)
            mv = stpool.tile([P, 2], f32, tag="mv", name="mv")
            nc.vector.bn_aggr(mv[:], st6[:])
            sd = stpool.tile([P, 1], f32, tag="sd", name="sd")
            nc.scalar.activation(sd[:], mv[:, 1:2], AF.Sqrt,
                                 bias=epsc[:], scale=1.0)
            rstd = stpool.tile([P, 1], f32, tag="rstd", name="rstd")
            nc.vector.reciprocal(rstd[:], sd[:])
            return mv, rstd

        def layer_norm(src_ap, dst_ap):
            mv, rstd = rstd_of(src_ap)
            nmr = stpool.tile([P, 1], f32, tag="nmr", name="nmr")
            nc.vector.scalar_tensor_tensor(
                nmr[:], mv[:, 0:1], -1.0, rstd[:], op0=ALU.mult, op1=ALU.mult)
            nc.scalar.activation(dst_ap, src_ap, AF.Identity,
                                 bias=nmr[:], scale=rstd[:])

        def transpose_v(m):
            # vT[d, t-block m] <- v[m][:, d] for both d-tiles (f32r transpose)
            for d in range(ND):
                tps = psS.tile([P, P], f32r, tag="psS", name="tvps")
                nc.tensor.transpose(tps[:], v_cur[m][:, ts(d, P)], identr[:])
                nc.scalar.copy(vT_sb[d][:, ts(m, P)], tps[:].bitcast(f32))

        def emit_x_half(h):
            # x[:, half h] = relu(Dx^T @ vT[:, h]) for both heads, then RoPE
            # for the whole head as 6 wide [P,4,HW] DVE ops. Relu evictions
            # alternate ACT/DVE to balance the two engines.
            c0 = h * HW
            for j in range(2):
                for mm in range(NDh):
                    xps = psS.tile([P, HW], f32, tag="psS", name="xps")
                    for k in range(ND):
                        nc.tensor.matmul(
                            xps[:],
                            dx_sb[2 * j + k][:, ts(mm, P)],
                            vT_sb[k][:, c0:c0 + HW],
                            start=(k == 0), stop=(k == ND - 1))
                    dst = xT[j][:, mm, c0:c0 + HW]
                    if mm % 2 == 0:
                        nc.scalar.activation(dst, xps[:], AF.Relu)
                    else:
                        nc.vector.tensor_scalar_max(dst, xps[:], 0.0)
                lo = xT[j][:, 0:4, c0:c0 + HW]
                hi = xT[j][:, 4:8, c0:c0 + HW]
                cm = cos_mg[:, :, c0:c0 + HW]
                sm = sin_mg[:, :, c0:c0 + HW]
                xrl = xr[j][:, 0:4, c0:c0 + HW]
                xrh = xr[j][:, 4:8, c0:c0 + HW]
                t1 = lnpool.tile([P, 4, HW], bf16, tag="ropet", name="rt1")
                nc.vector.tensor_mul(t1[:], hi, sm)
                nc.vector.tensor_mul(xrl, lo, cm)
                nc.vector.tensor_sub(xrl, xrl, t1[:])
                t2 = lnpool.tile([P, 4, HW], bf16, tag="ropet", name="rt2")
                nc.vector.tensor_mul(t2[:], lo, sm)
                nc.vector.tensor_mul(xrh, hi, cm)
                nc.vector.tensor_add(xrh, xrh, t2[:])

        def emit_ln_blocks(ms, zr, coff):
            # v[m] = LN(v[m] + LN(z[m])) for the given token blocks, into
            # fresh ring slots. The inner LN's mean-subtraction is dropped:
            # the outer LN is invariant to a per-row constant shift, so
            # LN(v + LN(z)) == LN(v + z*rstd_z).
            for i, m in enumerate(ms):
                zb = zr[:, coff + i * D:coff + (i + 1) * D]
                _, rstd_z = rstd_of(zb)
                w = lnpool.tile([P, D], f32, tag="w", name=f"w{m}")
                nc.vector.scalar_tensor_tensor(
                    w[:], zb, rstd_z[:], v_cur[m][:].bitcast(f32),
                    op0=ALU.mult, op1=ALU.add)
                vn = vpool.tile([P, D], f32r, tag=f"v{m}", name=f"v{m}")
                layer_norm(w[:], vn[:])
                v_cur[m] = vn

        def emit_ln_half(h, zr, with_tv=True):
            emit_ln_blocks(range(NHB * h, NHB * h + NHB), zr, 0)
            if with_tv:
                for mm in range(NHB):
                    transpose_v(NHB * h + mm)

        def emit_s_rows(h, j, k_lo, k_hi, s_tiles):
            # S row blocks [k_lo, k_hi) for head j, columns of half h.
            # Row block k only needs xr columns of block k as lhsT, so rows
            # 0..3 can run before this layer's half-1 x/RoPE exist.
            c0 = h * HW
            for k in range(k_lo, k_hi):
                sps = psS.tile([P, HW], f32, tag="psS", name="sps")
                for kk in range(NDh):
                    nc.tensor.matmul(
                        sps[:],
                        xr[j][:, kk, ts(k, P)],
                        xr[j][:, kk, c0:c0 + HW],
                        start=(kk == 0), stop=(kk == NDh - 1))
                s_sb = spool.tile([P, HW], f32r, tag="score", name=f"s{k}")
                nc.scalar.copy(s_sb[:], sps[:])
                s_tiles.append(s_sb)

        def emit_ayz(h, j, z_ps, v_l, s_tiles):
            # aT = v^T @ S (uses S's symmetry), y = relu(Dy^T aT) * x, and
            # z += E_h^T y accumulating across both heads in PSUM.
            c0 = h * HW
            aT_ps = [psA.tile([P, HW], f32, tag="psA", name=f"aTps{m}")
                     for m in range(ND)]
            for k in range(NT):
                for m in range(ND):
                    nc.tensor.matmul(
                        aT_ps[m][:], v_l[k][:, ts(m, P)], s_tiles[k][:],
                        start=(k == 0), stop=(k == NT - 1))
            aT = []
            for m in range(ND):
                at = apool.tile([P, HW], bf16, tag="aT", name=f"aT{m}")
                nc.vector.tensor_copy(at[:], aT_ps[m][:])
                aT.append(at)
            y_tiles = []
            for k in range(NDh):
                yps = psY.tile([P, HW], f32, tag="psY", name="yps")
                for kk in range(ND):
                    nc.tensor.matmul(
                        yps[:],
                        dy_sb[2 * j + kk][:, ts(k, P)],
                        aT[kk][:],
                        start=(kk == 0), stop=(kk == ND - 1))
                y_sb = ypool.tile([P, HW], bf16, tag="yT", name=f"y{k}")
                nc.vector.scalar_tensor_tensor(
                    y_sb[:], yps[:], 0.0, xT[j][:, k, c0:c0 + HW],
                    op0=ALU.max, op1=ALU.mult)
                y_tiles.append(y_sb)
                if k > 0:
                    for m in range(ND):
                        nc.tensor.matmul(
                            z_ps[m][:], eh_sb[NDh * j + k - 1][:, ts(m, P)],
                            y_tiles[k - 1][:],
                            start=(j == 0 and k - 1 == 0), stop=False)
            for m in range(ND):
                nc.tensor.matmul(
                    z_ps[m][:], eh_sb[NDh * j + NDh - 1][:, ts(m, P)],
                    y_tiles[NDh - 1][:],
                    start=False, stop=(j == 1))

        def emit_attn_half(h, j, z_ps, v_l, layer):
            s_tiles = []
            emit_s_rows(h, j, 0, NT, s_tiles)
            emit_ayz(h, j, z_ps, v_l, s_tiles)

        def emit_z_allreduce(h, z_ps, layer, nsplit=1):
            # Evict z (bf16), transpose to [T-half, D] packed layout, DMA to
            # DRAM, AllReduce over the core pair, DMA the sum back. With
            # nsplit>1 the message is split into independent ARs so the
            # consumer can pipeline against them (used for the final layer).
            z_sb = [zpool.tile([P, HW], bf16, tag=f"zsb{m}",
                               name=f"zsb{m}_{layer}_{h}") for m in range(ND)]
            for m in range(ND):
                nc.vector.tensor_copy(z_sb[m][:], z_ps[m][:])
            zq = zpool.tile([P, T], bf16, tag="zq", name=f"zq{layer}_{h}")
            for mm in range(NHB):
                for kd in range(ND):
                    tzp = psY.tile([P, P], bf16, tag="psY", name="tzp")
                    nc.tensor.transpose(
                        tzp[:], z_sb[kd][:, ts(mm, P)], identb[:])
                    nc.scalar.copy(
                        zq[:, mm * D + kd * P:mm * D + (kd + 1) * P], tzp[:])
            zrs = []
            w = T // nsplit
            for s in range(nsplit):
                zin = dpool.tile([P, w], bf16, tag=f"zin{h}_{s}",
                                 name=f"zin{h}_{s}_{layer}")
                zout = dpool.tile([P, w], bf16, tag=f"zout{h}_{s}",
                                  name=f"zout{h}_{s}_{layer}")
                nc.sync.dma_start(zin[:], zq[:, s * w:(s + 1) * w])
                nc.gpsimd.collective_compute(
                    "AllReduce", mybir.AluOpType.add,
                    ins=[zin.opt()], outs=[zout.opt()],
                    replica_groups=rg)
                zr = zpool.tile([P, w], bf16, tag=f"zr{h}_{s}",
                                name=f"zr{h}_{s}_{layer}")
                nc.sync.dma_start(zr[:], zout[:])
                zrs.append(zr)
            return zrs[0] if nsplit == 1 else zrs

        # ---- embedding: v = LN(onehot @ emb_w) ----
        for m in range(NT):
            eps_t = psS.tile([P, D], f32, tag="psS", name="embps")
            for k in range(ND):
                nc.tensor.matmul(eps_t[:], oh_sb[k][:, ts(m, P)], ew_sb[k][:],
                                 start=(k == 0), stop=(k == ND - 1))
            emb_t = lnpool.tile([P, D], f32, tag="w", name="embt")
            nc.scalar.copy(emb_t[:], eps_t[:])
            layer_norm(emb_t[:], v_cur[m][:])
            transpose_v(m)

        # ---- layers, software-pipelined across the per-half AllReduces ----
        # Invariant entering layer l: x/RoPE for half 0 of layer l are
        # emitted; for l>0 the AR for half 1 of layer l-1 is in flight and
        # its LN update has NOT been emitted yet. S(half0) rows 0..3 (which
        # need only half-0 RoPE) are emitted before that LN so the PE has
        # AR-independent work while the collective + LN chain drain.
        emit_x_half(0)
        zr_pend = None  # half-1 AR result of the previous layer
        for layer in range(L):
            s00, s01 = [], []
            emit_s_rows(0, 0, 0, NHB, s00)
            emit_s_rows(0, 1, 0, NHB, s01)
            if zr_pend is not None:
                emit_ln_half(1, zr_pend)  # waits on the in-flight AR
            v_l = list(v_cur)  # layer l's v is complete only after the LN
            emit_x_half(1)
            emit_s_rows(0, 0, NHB, NT, s00)
            z_ps = [psZ.tile([P, HW], f32, tag="psZ", name=f"zps{m}")
                    for m in range(ND)]
            emit_ayz(0, 0, z_ps, v_l, s00)
            emit_s_rows(0, 1, NHB, NT, s01)
            emit_ayz(0, 1, z_ps, v_l, s01)
            zr0 = emit_z_allreduce(0, z_ps, layer)
            z_ps = [psZ.tile([P, HW], f32, tag="psZ", name=f"zps{m}b")
                    for m in range(ND)]
            emit_attn_half(1, 0, z_ps, v_l, layer)
            # AR(half0) has landed behind half1's work; vT refresh is
            # deferred to the layer end so the PE never waits on this LN.
            emit_ln_half(0, zr0, with_tv=False)
            emit_attn_half(1, 1, z_ps, v_l, layer)
            if layer < L - 1:
                zr_pend = emit_z_allreduce(1, z_ps, layer)
                for mm in range(NHB):
                    transpose_v(mm)
                emit_x_half(0)
            else:
                zr_q = emit_z_allreduce(1, z_ps, layer, nsplit=2)

        # ---- readout, pipelined against the final AR quarters ----
        def emit_readout_blocks(ms):
            for m in ms:
                rps = psS.tile([P, V], f32, tag="psS", name="rps")
                for k in range(ND):
                    nc.tensor.matmul(rps[:], vT_sb[k][:, ts(m, P)], ro_sb[k][:],
                                     start=(k == 0), stop=(k == ND - 1))
                o_sb = lnpool.tile([P, V], f32, tag="o", name=f"o{m}")
                nc.scalar.copy(o_sb[:], rps[:])
                nc.sync.dma_start(d_out[ts(m, P), :], o_sb[:])

        for mm in range(NHB):
            transpose_v(mm)
        emit_readout_blocks(range(0, 4))
        emit_ln_blocks([4, 5], zr_q[0], 0)
        for m in (4, 5):
            transpose_v(m)
        emit_readout_blocks([4, 5])
        emit_ln_blocks([6, 7], zr_q[1], 0)
        for m in (6, 7):
            transpose_v(m)
        emit_readout_blocks([6, 7])

    nc.compile()
    return nc


def _get_program():
    if "nc" not in _CACHE:
        _CACHE["nc"] = _build_program()
    return _CACHE["nc"]


def _rope_tables():
    inv = (1.0 / (10000.0 ** (np.arange(0, Dh, 2, dtype=np.float32) / Dh)))
    tt = np.arange(T, dtype=np.float32)
    freqs = np.outer(tt, inv).astype(np.float32)  # [T, Dh/2]
    cosT = np.ascontiguousarray(np.cos(freqs).T, dtype=np.float32)
    sinT = np.ascontiguousarray(np.sin(freqs).T, dtype=np.float32)
    return cosT, sinT


def kernel(**inputs):
    global LAST_RESULT
    import ml_dtypes
    from concourse import bass_utils

    bf = ml_dtypes.bfloat16
    tokens = np.asarray(inputs["tokens"])
    emb_w = np.ascontiguousarray(inputs["emb_w"], dtype=np.float32)
    E = np.ascontiguousarray(inputs["E"], dtype=np.float32)
    Dx = np.ascontiguousarray(inputs["Dx"], dtype=np.float32)
    Dy = np.ascontiguousarray(inputs["Dy"], dtype=np.float32)
    readout = np.ascontiguousarray(inputs["readout"], dtype=np.float32)

    cosT, sinT = _rope_tables()

    in_maps = []
    for c in range(NCORES):
        b, hp = c // 2, c % 2
        oh = np.zeros((V, T), dtype=bf)
        oh[np.asarray(tokens[b], dtype=np.int64), np.arange(T)] = 1.0
        in_maps.append({
            "onehotT": oh,
            "emb_w": emb_w.astype(bf),
            "dx": np.ascontiguousarray(
                Dx[2 * hp:2 * hp + 2].reshape(2 * D, Dh).astype(bf)),
            "dy": np.ascontiguousarray(
                Dy[2 * hp:2 * hp + 2].reshape(2 * D, Dh).astype(bf)),
            "eh": np.ascontiguousarray(
                E[2 * hp * Dh:(2 * hp + 2) * Dh].astype(bf)),
            "cosT": cosT.astype(bf),
            "sinT": sinT.astype(bf),
            "readout": readout.astype(bf),
        })

    nc = _get_program()
    res = bass_utils.run_bass_kernel_spmd(
        nc, in_maps, core_ids=list(range(NCORES)),
        trace=bool(int(os.environ.get("KERNEL_TRACE", "0"))))
    LAST_RESULT = res
    out = np.stack([res.results[2 * b]["out"] for b in range(B)], axis=0)
    return out


# revision 42
# speedup vs baseline: 1.5670x; 1.0675x over previous
# Trainium2 Bass kernel for nn_BDH_66056597013022 (dense_transformer).
#
# Model (per reference):
#   v = LN(emb_w[tokens])                                  [B,T,D]
#   6x: x  = relu(v @ Dx_h)            per head            [B,H,T,Dh]
#       xr = RoPE(x)
#       S  = xr @ xr^T                 (no softmax)        [B,H,T,T]
#       a  = S @ v                                         [B,H,T,D]
#       y  = relu(a @ Dy_h) * x                            [B,H,T,Dh]
#       v  = LN(v + LN(concat_h(y) @ E))
#   out = v @ readout                                      [B,T,V]
#
# Shapes: B=4 T=1024 H=4 N=4096 D=256 L=6 V=256, Dh=N/H=1024.
#
# Sharding (8 cores): core c -> batch b=c//2, head-pair hp=c%2 (heads 2hp,2hp+1).
# The only cross-core coupling is the head-sum in z = y @ E, reduced with a
# 2-rank AllReduce between cores {2b,2b+1}; both cores then redundantly do the
# LN/v update. Even cores' outputs are returned.
#
# Precision: weights and most activations (x, xr, a, y, z, vT) are bf16
# (PE streams bf16 at 1 cyc/row like f32r; DVE gets the 2x packed mode for
# RoPE; SBUF/DMA halve). v and S stay f32r (the PE forbids mixing 32-bit and
# 16-bit matmul operands, so the a = S @ v contraction pairs f32r with f32r).
# PSUM accumulation is fp32 everywhere. Simulated end-to-end rel err ~1e-2
# vs the 2e-2 gate.
#
# Schedule (the point of this rewrite): attention is computed per
# token-column half (512 cols), z accumulates in PSUM across both local
# heads, and each half's AllReduce is launched as soon as that half's z is
# done. AR(half0) hides under half1's S/a/y/z; AR(half1) hides under the
# next layer's x+RoPE on half0, which only need half0's updated v. The LN
# v-update for a half is emitted at a point where its AR has surely landed.
# This removes the baseline's keep_pe_warm filler and its ~30us/layer of PE
# idle at RoPE and the layer boundary.

import os
import numpy as np

B, T, H, N, D, L, V = 4, 1024, 4, 4096, 256, 6, 256
Dh = N // H
EPS = 1e-5
NCORES = 8
P = 128
NT = T // P    # 8 token blocks
ND = D // P    # 2 model-dim blocks
NDh = Dh // P  # 8 head-dim blocks
HW = 512       # half width (token columns)
NHB = 4        # token blocks per half

_CACHE = {}
LAST_RESULT = None


def _build_program():
    from contextlib import ExitStack

    import concourse.bass as bass
    import concourse.bacc as bacc
    import concourse.tile as tile
    import concourse.mybir as mybir
    from concourse.masks import make_identity

    f32 = mybir.dt.float32
    f32r = mybir.dt.float32r
    bf16 = mybir.dt.bfloat16
    AF = mybir.ActivationFunctionType
    ALU = mybir.AluOpType
    ts = bass.ts

    nc = bacc.Bacc("TRN2", target_bir_lowering=False, debug=False,
                   enable_asserts=False, num_devices=NCORES)

    d_oh = nc.dram_tensor("onehotT", [V, T], bf16, kind="ExternalInput").ap()
    d_ew = nc.dram_tensor("emb_w", [V, D], bf16, kind="ExternalInput").ap()
    d_dx = nc.dram_tensor("dx", [2 * D, Dh], bf16, kind="ExternalInput").ap()
    d_dy = nc.dram_tensor("dy", [2 * D, Dh], bf16, kind="ExternalInput").ap()
    d_eh = nc.dram_tensor("eh", [2 * Dh, D], bf16, kind="ExternalInput").ap()
    d_cos = nc.dram_tensor("cosT", [Dh // 2, T], bf16, kind="ExternalInput").ap()
    d_sin = nc.dram_tensor("sinT", [Dh // 2, T], bf16, kind="ExternalInput").ap()
    d_ro = nc.dram_tensor("readout", [D, V], bf16, kind="ExternalInput").ap()
    d_out = nc.dram_tensor("out", [T, V], f32, kind="ExternalOutput").ap()

    rg = [[0, 1], [2, 3], [4, 5], [6, 7]]

    with tile.TileContext(nc) as tc, ExitStack() as ctx:
        wpool = ctx.enter_context(tc.tile_pool(name="weights", bufs=1))
        vpool = ctx.enter_context(tc.tile_pool(name="vpool", bufs=2))
        xpool = ctx.enter_context(tc.tile_pool(name="xpool", bufs=1))
        xrpool = ctx.enter_context(tc.tile_pool(name="xrpool", bufs=1))
        spool = ctx.enter_context(tc.tile_pool(name="spool", bufs=12))
        apool = ctx.enter_context(tc.tile_pool(name="apool", bufs=4))
        ypool = ctx.enter_context(tc.tile_pool(name="ypool", bufs=3))
        zpool = ctx.enter_context(tc.tile_pool(name="zpool", bufs=2))
        lnpool = ctx.enter_context(tc.tile_pool(name="lnpool", bufs=3))
        stpool = ctx.enter_context(tc.tile_pool(name="stpool", bufs=4))
        psS = ctx.enter_context(tc.tile_pool(name="psS", bufs=2, space="PSUM"))
        psA = ctx.enter_context(tc.tile_pool(name="psA", bufs=2, space="PSUM"))
        psY = ctx.enter_context(tc.tile_pool(name="psY", bufs=2, space="PSUM"))
        psZ = ctx.enter_context(tc.tile_pool(name="psZ", bufs=2, space="PSUM"))
        dpool = ctx.enter_context(tc.tile_pool(name="drampool", bufs=2, space="DRAM"))

        # ---- persistent weights (all bf16, DMA straight in) ----
        def load_bf16(dram_ap, n_tiles, width, tag):
            tiles = []
            for i in range(n_tiles):
                wt = wpool.tile([P, width], bf16, tag=f"{tag}{i}", name=f"{tag}{i}")
                nc.sync.dma_start(wt[:], dram_ap[ts(i, P), :])
                tiles.append(wt)
            return tiles

        oh_sb = load_bf16(d_oh, ND, T, "oh")
        ew_sb = load_bf16(d_ew, ND, D, "ew")
        dx_sb = load_bf16(d_dx, 2 * ND, Dh, "dx")
        # cos/sin packed as [P, 4, T] so RoPE can run as wide multi-block ops
        cos_mg = wpool.tile([P, 4, T], bf16, tag="cosmg", name="cosmg")
        sin_mg = wpool.tile([P, 4, T], bf16, tag="sinmg", name="sinmg")
        for i in range(4):
            nc.sync.dma_start(cos_mg[:, i, :], d_cos[ts(i, P), :])
        for i in range(4):
            nc.sync.dma_start(sin_mg[:, i, :], d_sin[ts(i, P), :])
        dy_sb = load_bf16(d_dy, 2 * ND, Dh, "dy")
        eh_sb = load_bf16(d_eh, 2 * NDh, D, "eh")
        ro_sb = load_bf16(d_ro, ND, V, "ro")

        ident = wpool.tile([P, P], f32, tag="ident", name="ident")
        make_identity(nc, ident)
        identr = wpool.tile([P, P], f32r, tag="identr", name="identr")
        nc.scalar.copy(identr[:], ident[:])
        identb = wpool.tile([P, P], bf16, tag="identb", name="identb")
        nc.scalar.copy(identb[:], ident[:])
        epsc = wpool.tile([P, 1], f32, tag="epsc", name="epsc")
        nc.gpsimd.memset(epsc[:], EPS)

        # Fire a tiny AllReduce during the weight DMAs so the first real
        # collective doesn't pay the ncfw warm-up latency.
        warm_sb = zpool.tile([P, 16], f32, tag="warmsb", name="warmsb")
        nc.gpsimd.memset(warm_sb[:], 0.0)
        warm_in = dpool.tile([P, 16], f32, tag="warmin", name="warmin")
        warm_out = dpool.tile([P, 16], f32, tag="warmout", name="warmout")
        nc.sync.dma_start(warm_in[:], warm_sb[:])
        nc.gpsimd.collective_compute(
            "AllReduce", mybir.AluOpType.add,
            ins=[warm_in.opt()], outs=[warm_out.opt()], replica_groups=rg)

        # ---- persistent activations ----
        # v is generation-buffered (ring of 2 per token block): the half-0 LN
        # update for layer l+1 is emitted while head 1 of layer l still reads
        # layer l's v, so updates go to a fresh ring slot and v_cur is
        # repointed. Attention snapshots v_cur at layer start.
        v_cur = [vpool.tile([P, D], f32r, tag=f"v{m}", name=f"v{m}")
                 for m in range(NT)]
        vT_sb = [vpool.tile([P, T], bf16, tag=f"vT{k}", name=f"vT{k}")
                 for k in range(ND)]
        # x and xr packed [P, NDh, T] per head so RoPE runs as 6 wide ops per
        # (head, half) — the ~230-cycle DVE per-op overhead amortizes 4x.
        xT = [xpool.tile([P, NDh, T], bf16, tag=f"x{j}", name=f"x{j}")
              for j in range(2)]
        xr = [xrpool.tile([P, NDh, T], bf16, tag=f"xr{j}", name=f"xr{j}")
              for j in range(2)]

        def rstd_of(src_ap):
            # 1/sqrt(var(src) + eps) as a [P,1] per-partition vector
            st6 = stpool.tile([P, 6], f32, tag="st6", name="st6")
            nc.vector.bn_stats(st6[:], src_ap)
            mv = stpool.tile([P, 2], f32, tag="mv", name="mv")
            nc.vector.bn_aggr(mv[:], st6[:])
            sd = stpool.tile([P, 1], f32, tag="sd", name="sd")
            nc.scalar.activation(sd[:], mv[:, 1:2], AF.Sqrt,
                                 bias=epsc[:], scale=1.0)
            rstd = stpool.tile([P, 1], f32, tag="rstd", name="rstd")
            nc.vector.reciprocal(rstd[:], sd[:])
            return mv, rstd

        def layer_norm(src_ap, dst_ap):
            mv, rstd = rstd_of(src_ap)
            nmr = stpool.tile([P, 1], f32, tag="nmr", name="nmr")
            nc.vector.scalar_tensor_tensor(
                nmr[:], mv[:, 0:1], -1.0, rstd[:], op0=ALU.mult, op1=ALU.mult)
            nc.scalar.activation(dst_ap, src_ap, AF.Identity,
                                 bias=nmr[:], scale=rstd[:])

        def transpose_v(m):
            # vT[d, t-block m] <- v[m][:, d] for both d-tiles (f32r transpose)
            for d in range(ND):
                tps = psS.tile([P, P], f32r, tag="psS", name="tvps")
                nc.tensor.transpose(tps[:], v_cur[m][:, ts(d, P)], identr[:])
                if d == 0:
                    nc.scalar.copy(vT_sb[d][:, ts(m, P)], tps[:].bitcast(f32))
                else:
                    nc.vector.tensor_copy(vT_sb[d][:, ts(m, P)],
                                          tps[:].bitcast(f32))

        def emit_x_half(h):
            # x[:, half h] = relu(Dx^T @ vT[:, h]) for both heads, then RoPE
            # for the whole head as 6 wide [P,4,HW] DVE ops. Relu evictions
            # alternate ACT/DVE to balance the two engines.
            c0 = h * HW
            for j in range(2):
                for mm in range(NDh):
                    xps = psS.tile([P, HW], f32, tag="psS", name="xps")
                    for k in range(ND):
                        nc.tensor.matmul(
                            xps[:],
                            dx_sb[2 * j + k][:, ts(mm, P)],
                            vT_sb[k][:, c0:c0 + HW],
                            start=(k == 0), stop=(k == ND - 1))
                    nc.scalar.activation(
                        xT[j][:, mm, c0:c0 + HW], xps[:], AF.Relu)
                lo = xT[j][:, 0:4, c0:c0 + HW]
                hi = xT[j][:, 4:8, c0:c0 + HW]
                cm = cos_mg[:, :, c0:c0 + HW]
                sm = sin_mg[:, :, c0:c0 + HW]
                xrl = xr[j][:, 0:4, c0:c0 + HW]
                xrh = xr[j][:, 4:8, c0:c0 + HW]
                t1 = lnpool.tile([P, 4, HW], bf16, tag="ropet", name="rt1")
                nc.vector.tensor_mul(t1[:], hi, sm)
                nc.vector.tensor_mul(xrl, lo, cm)
                nc.vector.tensor_sub(xrl, xrl, t1[:])
                t2 = lnpool.tile([P, 4, HW], bf16, tag="ropet", name="rt2")
                nc.vector.tensor_mul(t2[:], lo, sm)
                nc.vector.tensor_mul(xrh, hi, cm)
                nc.vector.tensor_add(xrh, xrh, t2[:])

        def emit_ln_blocks(ms, zr, coff):
            # v[m] = LN(v[m] + LN(z[m])) for the given token blocks, into
            # fresh ring slots. The inner LN's mean-subtraction is dropped:
            # the outer LN is invariant to a per-row constant shift, so
            # LN(v + LN(z)) == LN(v + z*rstd_z).
            for i, m in enumerate(ms):
                zb = zr[:, coff + i * D:coff + (i + 1) * D]
                _, rstd_z = rstd_of(zb)
                w = lnpool.tile([P, D], f32, tag="w", name=f"w{m}")
                nc.vector.scalar_tensor_tensor(
                    w[:], zb, rstd_z[:], v_cur[m][:].bitcast(f32),
                    op0=ALU.mult, op1=ALU.add)
                vn = vpool.tile([P, D], f32r, tag=f"v{m}", name=f"v{m}")
                layer_norm(w[:], vn[:])
                v_cur[m] = vn

        def emit_ln_half(h, zr, with_tv=True):
            emit_ln_blocks(range(NHB * h, NHB * h + NHB), zr, 0)
            if with_tv:
                for mm in range(NHB):
                    transpose_v(NHB * h + mm)

        def emit_s_rows(h, j, k_lo, k_hi, s_tiles):
            # S row blocks [k_lo, k_hi) for head j, columns of half h.
            # Row block k only needs xr columns of block k as lhsT, so rows
            # 0..3 can run before this layer's half-1 x/RoPE exist.
            c0 = h * HW
            for k in range(k_lo, k_hi):
                sps = psS.tile([P, HW], f32, tag="psS", name="sps")
                for kk in range(NDh):
                    nc.tensor.matmul(
                        sps[:],
                        xr[j][:, kk, ts(k, P)],
                        xr[j][:, kk, c0:c0 + HW],
                        start=(kk == 0), stop=(kk == NDh - 1))
                s_sb = spool.tile([P, HW], f32r, tag="score", name=f"s{k}")
                # alternate eviction engine so the 2-deep PSUM ring never
                # waits on a busy engine
                if k % 2 == 0:
                    nc.scalar.copy(s_sb[:], sps[:])
                else:
                    nc.vector.tensor_copy(s_sb[:], sps[:])
                s_tiles.append(s_sb)

        def emit_ayz(h, j, z_ps, v_l, s_tiles):
            # aT = v^T @ S (uses S's symmetry), y = relu(Dy^T aT) * x, and
            # z += E_h^T y accumulating across both heads in PSUM.
            c0 = h * HW
            aT_ps = [psA.tile([P, HW], f32, tag="psA", name=f"aTps{m}")
                     for m in range(ND)]
            for k in range(NT):
                for m in range(ND):
                    nc.tensor.matmul(
                        aT_ps[m][:], v_l[k][:, ts(m, P)], s_tiles[k][:],
                        start=(k == 0), stop=(k == NT - 1))
            aT = []
            for m in range(ND):
                at = apool.tile([P, HW], bf16, tag="aT", name=f"aT{m}")
                nc.vector.tensor_copy(at[:], aT_ps[m][:])
                aT.append(at)
            y_tiles = []
            for k in range(NDh):
                yps = psY.tile([P, HW], f32, tag="psY", name="yps")
                for kk in range(ND):
                    nc.tensor.matmul(
                        yps[:],
                        dy_sb[2 * j + kk][:, ts(k, P)],
                        aT[kk][:],
                        start=(kk == 0), stop=(kk == ND - 1))
                y_sb = ypool.tile([P, HW], bf16, tag="yT", name=f"y{k}")
                nc.vector.scalar_tensor_tensor(
                    y_sb[:], yps[:], 0.0, xT[j][:, k, c0:c0 + HW],
                    op0=ALU.max, op1=ALU.mult)
                y_tiles.append(y_sb)
                if k > 0:
                    for m in range(ND):
                        nc.tensor.matmul(
                            z_ps[m][:], eh_sb[NDh * j + k - 1][:, ts(m, P)],
                            y_tiles[k - 1][:],
                            start=(j == 0 and k - 1 == 0), stop=False)
            for m in range(ND):
                nc.tensor.matmul(
                    z_ps[m][:], eh_sb[NDh * j + NDh - 1][:, ts(m, P)],
                    y_tiles[NDh - 1][:],
                    start=False, stop=(j == 1))

        def emit_attn_half(h, j, z_ps, v_l, layer):
            s_tiles = []
            emit_s_rows(h, j, 0, NT, s_tiles)
            emit_ayz(h, j, z_ps, v_l, s_tiles)

        def emit_z_allreduce(h, z_ps, layer, nsplit=1):
            # Evict z (bf16), transpose to [T-half, D] packed layout, DMA to
            # DRAM, AllReduce over the core pair, DMA the sum back. With
            # nsplit>1 the message is split into independent ARs so the
            # consumer can pipeline against them (used for the final layer).
            z_sb = [zpool.tile([P, HW], bf16, tag=f"zsb{m}",
                               name=f"zsb{m}_{layer}_{h}") for m in range(ND)]
            for m in range(ND):
                nc.vector.tensor_copy(z_sb[m][:], z_ps[m][:])
            zq = zpool.tile([P, T], bf16, tag="zq", name=f"zq{layer}_{h}")
            for mm in range(NHB):
                for kd in range(ND):
                    tzp = psY.tile([P, P], bf16, tag="psY", name="tzp")
                    nc.tensor.transpose(
                        tzp[:], z_sb[kd][:, ts(mm, P)], identb[:])
                    dst = zq[:, mm * D + kd * P:mm * D + (kd + 1) * P]
                    if kd == 0:
                        nc.scalar.copy(dst, tzp[:])
                    else:
                        nc.vector.tensor_copy(dst, tzp[:])
            zrs = []
            w = T // nsplit
            for s in range(nsplit):
                zin = dpool.tile([P, w], bf16, tag=f"zin{h}_{s}",
                                 name=f"zin{h}_{s}_{layer}")
                zout = dpool.tile([P, w], bf16, tag=f"zout{h}_{s}",
                                  name=f"zout{h}_{s}_{layer}")
                nc.sync.dma_start(zin[:], zq[:, s * w:(s + 1) * w])
                nc.gpsimd.collective_compute(
                    "AllReduce", mybir.AluOpType.add,
                    ins=[zin.opt()], outs=[zout.opt()],
                    replica_groups=rg)
                zr = zpool.tile([P, w], bf16, tag=f"zr{h}_{s}",
                                name=f"zr{h}_{s}_{layer}")
                nc.sync.dma_start(zr[:], zout[:])
                zrs.append(zr)
            return zrs[0] if nsplit == 1 else zrs

        # ---- embedding: v = LN(onehot @ emb_w) ----
        for m in range(NT):
            eps_t = psS.tile([P, D], f32, tag="psS", name="embps")
            for k in range(ND):
                nc.tensor.matmul(eps_t[:], oh_sb[k][:, ts(m, P)], ew_sb[k][:],
                                 start=(k == 0), stop=(k == ND - 1))
            emb_t = lnpool.tile([P, D], f32, tag="w", name="embt")
            nc.scalar.copy(emb_t[:], eps_t[:])
            layer_norm(emb_t[:], v_cur[m][:])
            transpose_v(m)

        # ---- layers, software-pipelined across the per-half AllReduces ----
        # Invariant entering layer l: x/RoPE for half 0 of layer l are
        # emitted; for l>0 the AR for half 1 of layer l-1 is in flight and
        # its LN update has NOT been emitted yet. S(half0) rows 0..3 (which
        # need only half-0 RoPE) are emitted before that LN so the PE has
        # AR-independent work while the collective + LN chain drain.
        emit_x_half(0)
        zr_pend = None  # half-1 AR result of the previous layer
        for layer in range(L):
            s00, s01 = [], []
            emit_s_rows(0, 0, 0, NHB, s00)
            emit_s_rows(0, 1, 0, NHB, s01)
            if zr_pend is not None:
                emit_ln_half(1, zr_pend)  # waits on the in-flight AR
            v_l = list(v_cur)  # layer l's v is complete only after the LN
            emit_x_half(1)
            emit_s_rows(0, 0, NHB, NT, s00)
            z_ps = [psZ.tile([P, HW], f32, tag="psZ", name=f"zps{m}")
                    for m in range(ND)]
            emit_ayz(0, 0, z_ps, v_l, s00)
            emit_s_rows(0, 1, NHB, NT, s01)
            emit_ayz(0, 1, z_ps, v_l, s01)
            zr0 = emit_z_allreduce(0, z_ps, layer)
            z_ps = [psZ.tile([P, HW], f32, tag="psZ", name=f"zps{m}b")
                    for m in range(ND)]
            emit_attn_half(1, 0, z_ps, v_l, layer)
            # AR(half0) has landed behind half1's work; vT refresh is
            # deferred to the layer end so the PE never waits on this LN.
            emit_ln_half(0, zr0, with_tv=False)
            emit_attn_half(1, 1, z_ps, v_l, layer)
            if layer < L - 1:
                zr_pend = emit_z_allreduce(1, z_ps, layer)
                for mm in range(NHB):
                    transpose_v(mm)
                emit_x_half(0)
            else:
                zr_q = emit_z_allreduce(1, z_ps, layer)

        # ---- readout, pipelined against the final AR quarters ----
        def emit_readout_blocks(ms):
            for m in ms:
                rps = psS.tile([P, V], f32, tag="psS", name="rps")
                for k in range(ND):
                    nc.tensor.matmul(rps[:], vT_sb[k][:, ts(m, P)], ro_sb[k][:],
                                     start=(k == 0), stop=(k == ND - 1))
                o_sb = lnpool.tile([P, V], f32, tag="o", name=f"o{m}")
                nc.scalar.copy(o_sb[:], rps[:])
                nc.sync.dma_start(d_out[ts(m, P), :], o_sb[:])

        for mm in range(NHB):
            transpose_v(mm)
        emit_readout_blocks(range(0, 4))
        emit_ln_blocks([4, 5], zr_q, 0)
        for m in (4, 5):
            transpose_v(m)
        emit_readout_blocks([4, 5])
        emit_ln_blocks([6, 7], zr_q, 2 * D)
        for m in (6, 7):
            transpose_v(m)
        emit_readout_blocks([6, 7])

    nc.compile()
    return nc


def _get_program():
    if "nc" not in _CACHE:
        _CACHE["nc"] = _build_program()
    return _CACHE["nc"]


def _rope_tables():
    inv = (1.0 / (10000.0 ** (np.arange(0, Dh, 2, dtype=np.float32) / Dh)))
    tt = np.arange(T, dtype=np.float32)
    freqs = np.outer(tt, inv).astype(np.float32)  # [T, Dh/2]
    cosT = np.ascontiguousarray(np.cos(freqs).T, dtype=np.float32)
    sinT = np.ascontiguousarray(np.sin(freqs).T, dtype=np.float32)
    return cosT, sinT


def kernel(**inputs):
    global LAST_RESULT
    import ml_dtypes
    from concourse import bass_utils

    bf = ml_dtypes.bfloat16
    tokens = np.asarray(inputs["tokens"])
    emb_w = np.ascontiguousarray(inputs["emb_w"], dtype=np.float32)
    E = np.ascontiguousarray(inputs["E"], dtype=np.float32)
    Dx = np.ascontiguousarray(inputs["Dx"], dtype=np.float32)
    Dy = np.ascontiguousarray(inputs["Dy"], dtype=np.float32)
    readout = np.ascontiguousarray(inputs["readout"], dtype=np.float32)

    cosT, sinT = _rope_tables()

    in_maps = []
    for c in range(NCORES):
        b, hp = c // 2, c % 2
        oh = np.zeros((V, T), dtype=bf)
        oh[np.asarray(tokens[b], dtype=np.int64), np.arange(T)] = 1.0
        in_maps.append({
            "onehotT": oh,
            "emb_w": emb_w.astype(bf),
            "dx": np.ascontiguousarray(
                Dx[2 * hp:2 * hp + 2].reshape(2 * D, Dh).astype(bf)),
            "dy": np.ascontiguousarray(
                Dy[2 * hp:2 * hp + 2].reshape(2 * D, Dh).astype(bf)),
            "eh": np.ascontiguousarray(
                E[2 * hp * Dh:(2 * hp + 2) * Dh].astype(bf)),
            "cosT": cosT.astype(bf),
            "sinT": sinT.astype(bf),
            "readout": readout.astype(bf),
        })

    nc = _get_program()
    res = bass_utils.run_bass_kernel_spmd(
        nc, in_maps, core_ids=list(range(NCORES)),
        trace=bool(int(os.environ.get("KERNEL_TRACE", "0"))))
    LAST_RESULT = res
    out = np.stack([res.results[2 * b]["out"] for b in range(B)], axis=0)
    return out


# revision 49
# speedup vs baseline: 1.6215x; 1.0347x over previous
# Trainium2 Bass kernel for nn_BDH_66056597013022 (dense_transformer).
#
# Model (per reference):
#   v = LN(emb_w[tokens])                                  [B,T,D]
#   6x: x  = relu(v @ Dx_h)            per head            [B,H,T,Dh]
#       xr = RoPE(x)
#       S  = xr @ xr^T                 (no softmax)        [B,H,T,T]
#       a  = S @ v                                         [B,H,T,D]
#       y  = relu(a @ Dy_h) * x                            [B,H,T,Dh]
#       v  = LN(v + LN(concat_h(y) @ E))
#   out = v @ readout                                      [B,T,V]
#
# Shapes: B=4 T=1024 H=4 N=4096 D=256 L=6 V=256, Dh=N/H=1024.
#
# Sharding (8 cores): core c -> batch b=c//2, head-pair hp=c%2 (heads 2hp,2hp+1).
# The only cross-core coupling is the head-sum in z = y @ E, reduced with a
# 2-rank AllReduce between cores {2b,2b+1}; both cores then redundantly do the
# LN/v update. Even cores' outputs are returned.
#
# Precision: weights and most activations (x, xr, a, y, z, vT) are bf16
# (PE streams bf16 at 1 cyc/row like f32r; DVE gets the 2x packed mode for
# RoPE; SBUF/DMA halve). v and S stay f32r (the PE forbids mixing 32-bit and
# 16-bit matmul operands, so the a = S @ v contraction pairs f32r with f32r).
# PSUM accumulation is fp32 everywhere. Simulated end-to-end rel err ~1e-2
# vs the 2e-2 gate.
#
# Schedule (the point of this rewrite): attention is computed per
# token-column half (512 cols), z accumulates in PSUM across both local
# heads, and each half's AllReduce is launched as soon as that half's z is
# done. AR(half0) hides under half1's S/a/y/z; AR(half1) hides under the
# next layer's x+RoPE on half0, which only need half0's updated v. The LN
# v-update for a half is emitted at a point where its AR has surely landed.
# This removes the baseline's keep_pe_warm filler and its ~30us/layer of PE
# idle at RoPE and the layer boundary.

import os
import numpy as np

B, T, H, N, D, L, V = 4, 1024, 4, 4096, 256, 6, 256
Dh = N // H
EPS = 1e-5
NCORES = 8
P = 128
NT = T // P    # 8 token blocks
ND = D // P    # 2 model-dim blocks
NDh = Dh // P  # 8 head-dim blocks
HW = 512       # half width (token columns)
NHB = 4        # token blocks per half

_CACHE = {}
LAST_RESULT = None


def _build_program():
    from contextlib import ExitStack

    import concourse.bass as bass
    import concourse.bacc as bacc
    import concourse.tile as tile
    import concourse.mybir as mybir
    from concourse.masks import make_identity

    f32 = mybir.dt.float32
    f32r = mybir.dt.float32r
    bf16 = mybir.dt.bfloat16
    AF = mybir.ActivationFunctionType
    ALU = mybir.AluOpType
    ts = bass.ts

    nc = bacc.Bacc("TRN2", target_bir_lowering=False, debug=False,
                   enable_asserts=False, num_devices=NCORES)

    d_oh = nc.dram_tensor("onehotT", [V, T], bf16, kind="ExternalInput").ap()
    d_ew = nc.dram_tensor("emb_w", [V, D], bf16, kind="ExternalInput").ap()
    d_dx = nc.dram_tensor("dx", [2 * D, Dh], bf16, kind="ExternalInput").ap()
    d_dy = nc.dram_tensor("dy", [2 * D, Dh], bf16, kind="ExternalInput").ap()
    d_eh = nc.dram_tensor("eh", [2 * Dh, D], bf16, kind="ExternalInput").ap()
    d_cos = nc.dram_tensor("cosT", [Dh // 2, T], bf16, kind="ExternalInput").ap()
    d_sin = nc.dram_tensor("sinT", [Dh // 2, T], bf16, kind="ExternalInput").ap()
    d_ro = nc.dram_tensor("readout", [D, V], bf16, kind="ExternalInput").ap()
    d_out = nc.dram_tensor("out", [T, V], f32, kind="ExternalOutput").ap()

    rg = [[0, 1], [2, 3], [4, 5], [6, 7]]

    with tile.TileContext(nc) as tc, ExitStack() as ctx:
        wpool = ctx.enter_context(tc.tile_pool(name="weights", bufs=1))
        vpool = ctx.enter_context(tc.tile_pool(name="vpool", bufs=2))
        xpool = ctx.enter_context(tc.tile_pool(name="xpool", bufs=1))
        xrpool = ctx.enter_context(tc.tile_pool(name="xrpool", bufs=1))
        spool = ctx.enter_context(tc.tile_pool(name="spool", bufs=12))
        apool = ctx.enter_context(tc.tile_pool(name="apool", bufs=4))
        ypool = ctx.enter_context(tc.tile_pool(name="ypool", bufs=3))
        zpool = ctx.enter_context(tc.tile_pool(name="zpool", bufs=2))
        lnpool = ctx.enter_context(tc.tile_pool(name="lnpool", bufs=3))
        rtpool = ctx.enter_context(tc.tile_pool(name="rtpool", bufs=2))
        lnw = ctx.enter_context(tc.tile_pool(name="lnw", bufs=8))
        stpool = ctx.enter_context(tc.tile_pool(name="stpool", bufs=4))
        psW = ctx.enter_context(tc.tile_pool(name="psW", bufs=4, space="PSUM"))
        psA = ctx.enter_context(tc.tile_pool(name="psA", bufs=2, space="PSUM"))
        psZ = ctx.enter_context(tc.tile_pool(name="psZ", bufs=2, space="PSUM"))
        dpool = ctx.enter_context(tc.tile_pool(name="drampool", bufs=2, space="DRAM"))

        # ---- persistent weights (all bf16, DMA straight in) ----
        def load_bf16(dram_ap, n_tiles, width, tag):
            tiles = []
            for i in range(n_tiles):
                wt = wpool.tile([P, width], bf16, tag=f"{tag}{i}", name=f"{tag}{i}")
                nc.sync.dma_start(wt[:], dram_ap[ts(i, P), :])
                tiles.append(wt)
            return tiles

        # onehot is only needed for the embedding matmuls — stage it through
        # the score ring rather than holding persistent SBUF.
        oh_sb = []
        for i in range(ND):
            t = spool.tile([P, T], bf16, tag="score", name=f"oh{i}")
            nc.sync.dma_start(t[:], d_oh[ts(i, P), :])
            oh_sb.append(t)
        ew_sb = load_bf16(d_ew, ND, D, "ew")
        dx_sb = load_bf16(d_dx, 2 * ND, Dh, "dx")
        # cos/sin packed as [P, 4, T] so RoPE can run as wide multi-block ops
        cos_mg = wpool.tile([P, 4, T], bf16, tag="cosmg", name="cosmg")
        sin_mg = wpool.tile([P, 4, T], bf16, tag="sinmg", name="sinmg")
        for i in range(4):
            nc.sync.dma_start(cos_mg[:, i, :], d_cos[ts(i, P), :])
        for i in range(4):
            nc.sync.dma_start(sin_mg[:, i, :], d_sin[ts(i, P), :])
        dy_sb = load_bf16(d_dy, 2 * ND, Dh, "dy")
        eh_sb = load_bf16(d_eh, 2 * NDh, D, "eh")
        ro_sb = load_bf16(d_ro, ND, V, "ro")

        ident = wpool.tile([P, P], f32, tag="ident", name="ident")
        make_identity(nc, ident)
        identr = wpool.tile([P, P], f32r, tag="identr", name="identr")
        nc.scalar.copy(identr[:], ident[:])
        identb = wpool.tile([P, P], bf16, tag="identb", name="identb")
        nc.scalar.copy(identb[:], ident[:])
        epsc = wpool.tile([P, 1], f32, tag="epsc", name="epsc")
        nc.gpsimd.memset(epsc[:], EPS)

        # Fire a tiny AllReduce during the weight DMAs so the first real
        # collective doesn't pay the ncfw warm-up latency.
        warm_sb = zpool.tile([P, 16], f32, tag="warmsb", name="warmsb")
        nc.gpsimd.memset(warm_sb[:], 0.0)
        warm_in = dpool.tile([P, 16], f32, tag="warmin", name="warmin")
        warm_out = dpool.tile([P, 16], f32, tag="warmout", name="warmout")
        nc.sync.dma_start(warm_in[:], warm_sb[:])
        nc.gpsimd.collective_compute(
            "AllReduce", mybir.AluOpType.add,
            ins=[warm_in.opt()], outs=[warm_out.opt()], replica_groups=rg)

        # ---- persistent activations ----
        # v is generation-buffered (ring of 2 per token block): the half-0 LN
        # update for layer l+1 is emitted while head 1 of layer l still reads
        # layer l's v, so updates go to a fresh ring slot and v_cur is
        # repointed. Attention snapshots v_cur at layer start.
        v_cur = [vpool.tile([P, D], f32r, tag=f"v{m}", name=f"v{m}")
                 for m in range(NT)]
        vT_sb = [vpool.tile([P, T], bf16, tag=f"vT{k}", name=f"vT{k}")
                 for k in range(ND)]
        # x and xr packed [P, NDh, T] per head so RoPE runs as 6 wide ops per
        # (head, half) — the ~230-cycle DVE per-op overhead amortizes 4x.
        xT = [xpool.tile([P, NDh, T], bf16, tag=f"x{j}", name=f"x{j}")
              for j in range(2)]
        xr = [xrpool.tile([P, NDh, T], bf16, tag=f"xr{j}", name=f"xr{j}")
              for j in range(2)]

        def rstd_of(src_ap):
            # 1/sqrt(var(src) + eps) as a [P,1] per-partition vector
            st6 = stpool.tile([P, 6], f32, tag="st6", name="st6")
            nc.vector.bn_stats(st6[:], src_ap)
            mv = stpool.tile([P, 2], f32, tag="mv", name="mv")
            nc.vector.bn_aggr(mv[:], st6[:])
            sd = stpool.tile([P, 1], f32, tag="sd", name="sd")
            nc.scalar.activation(sd[:], mv[:, 1:2], AF.Sqrt,
                                 bias=epsc[:], scale=1.0)
            rstd = stpool.tile([P, 1], f32, tag="rstd", name="rstd")
            nc.vector.reciprocal(rstd[:], sd[:])
            return mv, rstd

        def layer_norm(src_ap, dst_ap):
            mv, rstd = rstd_of(src_ap)
            nmr = stpool.tile([P, 1], f32, tag="nmr", name="nmr")
            nc.vector.scalar_tensor_tensor(
                nmr[:], mv[:, 0:1], -1.0, rstd[:], op0=ALU.mult, op1=ALU.mult)
            nc.scalar.activation(dst_ap, src_ap, AF.Identity,
                                 bias=nmr[:], scale=rstd[:])

        def transpose_v(m):
            # vT[d, t-block m] <- v[m][:, d] for both d-tiles (f32r transpose)
            for d in range(ND):
                tps = psW.tile([P, P], f32r, tag="psW", name="tvps")
                nc.tensor.transpose(tps[:], v_cur[m][:, ts(d, P)], identr[:])
                if d == 0:
                    nc.scalar.copy(vT_sb[d][:, ts(m, P)], tps[:].bitcast(f32))
                else:
                    nc.vector.tensor_copy(vT_sb[d][:, ts(m, P)],
                                          tps[:].bitcast(f32))

        def emit_x_half(h):
            # x[:, half h] = relu(Dx^T @ vT[:, h]) for both heads, then RoPE
            # for the whole head as 6 wide [P,4,HW] DVE ops. Relu evictions
            # alternate ACT/DVE to balance the two engines.
            c0 = h * HW
            for j in range(2):
                for mm in range(NDh):
                    xps = psW.tile([P, HW], f32, tag="psW", name="xps")
                    for k in range(ND):
                        nc.tensor.matmul(
                            xps[:],
                            dx_sb[2 * j + k][:, ts(mm, P)],
                            vT_sb[k][:, c0:c0 + HW],
                            start=(k == 0), stop=(k == ND - 1))
                    nc.scalar.activation(
                        xT[j][:, mm, c0:c0 + HW], xps[:], AF.Relu)
                lo = xT[j][:, 0:4, c0:c0 + HW]
                hi = xT[j][:, 4:8, c0:c0 + HW]
                cm = cos_mg[:, :, c0:c0 + HW]
                sm = sin_mg[:, :, c0:c0 + HW]
                xrl = xr[j][:, 0:4, c0:c0 + HW]
                xrh = xr[j][:, 4:8, c0:c0 + HW]
                t1 = rtpool.tile([P, 4, HW], bf16, tag="ropet", name="rt1")
                nc.vector.tensor_mul(t1[:], hi, sm)
                nc.vector.tensor_mul(xrl, lo, cm)
                nc.vector.tensor_sub(xrl, xrl, t1[:])
                t2 = rtpool.tile([P, 4, HW], bf16, tag="ropet", name="rt2")
                nc.vector.tensor_mul(t2[:], lo, sm)
                nc.vector.tensor_mul(xrh, hi, cm)
                nc.vector.tensor_add(xrh, xrh, t2[:])

        def emit_ln_blocks(ms, zr, coff):
            # v[m] = LN(v[m] + LN(z[m])) for the given token blocks, into
            # fresh ring slots. The inner LN's mean-subtraction is dropped:
            # the outer LN is invariant to a per-row constant shift, so
            # LN(v + LN(z)) == LN(v + z*rstd_z). Emitted stage-batched so
            # the DVE FIFO never stalls on a single block's ACT round-trip.
            n = len(ms)
            zbs = [zr[:, coff + i * D:coff + (i + 1) * D] for i in range(n)]

            def stats(srcs):
                mvs = []
                for i in range(n):
                    st6 = stpool.tile([P, 6], f32, tag="st6", name="st6")
                    nc.vector.bn_stats(st6[:], srcs[i])
                    mv = stpool.tile([P, 2], f32, tag="mv", name="mv")
                    nc.vector.bn_aggr(mv[:], st6[:])
                    mvs.append(mv)
                sds = []
                for i in range(n):
                    sd = stpool.tile([P, 1], f32, tag="sd", name="sd")
                    nc.scalar.activation(sd[:], mvs[i][:, 1:2], AF.Sqrt,
                                         bias=epsc[:], scale=1.0)
                    sds.append(sd)
                rstds = []
                for i in range(n):
                    rstd = stpool.tile([P, 1], f32, tag="rstd", name="rstd")
                    nc.vector.reciprocal(rstd[:], sds[i][:])
                    rstds.append(rstd)
                return mvs, rstds

            _, rstds_z = stats(zbs)
            ws = []
            for i, m in enumerate(ms):
                w = lnw.tile([P, D], f32, tag="w", name=f"w{m}")
                nc.vector.scalar_tensor_tensor(
                    w[:], zbs[i], rstds_z[i][:], v_cur[m][:].bitcast(f32),
                    op0=ALU.mult, op1=ALU.add)
                ws.append(w)
            mvs_w, rstds_w = stats([w[:] for w in ws])
            nmrs = []
            for i in range(n):
                nmr = stpool.tile([P, 1], f32, tag="nmr", name="nmr")
                nc.vector.scalar_tensor_tensor(
                    nmr[:], mvs_w[i][:, 0:1], -1.0, rstds_w[i][:],
                    op0=ALU.mult, op1=ALU.mult)
                nmrs.append(nmr)
            for i, m in enumerate(ms):
                vn = vpool.tile([P, D], f32r, tag=f"v{m}", name=f"v{m}")
                nc.scalar.activation(vn[:], ws[i][:], AF.Identity,
                                     bias=nmrs[i][:], scale=rstds_w[i][:])
                v_cur[m] = vn

        def emit_ln_half(h, zr, with_tv=True):
            emit_ln_blocks(range(NHB * h, NHB * h + NHB), zr, 0)
            if with_tv:
                for mm in range(NHB):
                    transpose_v(NHB * h + mm)

        def emit_s_rows(h, j, k_lo, k_hi, s_tiles):
            # S row blocks [k_lo, k_hi) for head j, columns of half h.
            # Row block k only needs xr columns of block k as lhsT, so rows
            # 0..3 can run before this layer's half-1 x/RoPE exist.
            c0 = h * HW
            for k in range(k_lo, k_hi):
                sps = psW.tile([P, HW], f32, tag="psW", name="sps")
                for kk in range(NDh):
                    nc.tensor.matmul(
                        sps[:],
                        xr[j][:, kk, ts(k, P)],
                        xr[j][:, kk, c0:c0 + HW],
                        start=(kk == 0), stop=(kk == NDh - 1))
                s_sb = spool.tile([P, HW], f32r, tag="score", name=f"s{k}")
                # alternate eviction engine so the 2-deep PSUM ring never
                # waits on a busy engine
                if k % 2 == 0:
                    nc.scalar.copy(s_sb[:], sps[:])
                else:
                    nc.vector.tensor_copy(s_sb[:], sps[:])
                s_tiles.append(s_sb)

        def emit_ayz(h, j, z_ps, v_l, s_tiles):
            # aT = v^T @ S (uses S's symmetry), y = relu(Dy^T aT) * x, and
            # z += E_h^T y accumulating across both heads in PSUM.
            c0 = h * HW
            aT_ps = [psA.tile([P, HW], f32, tag="psA", name=f"aTps{m}")
                     for m in range(ND)]
            for k in range(NT):
                for m in range(ND):
                    nc.tensor.matmul(
                        aT_ps[m][:], v_l[k][:, ts(m, P)], s_tiles[k][:],
                        start=(k == 0), stop=(k == NT - 1))
            aT = []
            for m in range(ND):
                at = apool.tile([P, HW], bf16, tag="aT", name=f"aT{m}")
                nc.vector.tensor_copy(at[:], aT_ps[m][:])
                aT.append(at)
            y_tiles = []
            for k in range(NDh):
                yps = psW.tile([P, HW], f32, tag="psW", name="yps")
                for kk in range(ND):
                    nc.tensor.matmul(
                        yps[:],
                        dy_sb[2 * j + kk][:, ts(k, P)],
                        aT[kk][:],
                        start=(kk == 0), stop=(kk == ND - 1))
                y_sb = ypool.tile([P, HW], bf16, tag="yT", name=f"y{k}")
                nc.vector.scalar_tensor_tensor(
                    y_sb[:], yps[:], 0.0, xT[j][:, k, c0:c0 + HW],
                    op0=ALU.max, op1=ALU.mult)
                y_tiles.append(y_sb)
                if k > 0:
                    for m in range(ND):
                        nc.tensor.matmul(
                            z_ps[m][:], eh_sb[NDh * j + k - 1][:, ts(m, P)],
                            y_tiles[k - 1][:],
                            start=(j == 0 and k - 1 == 0), stop=False)
            for m in range(ND):
                nc.tensor.matmul(
                    z_ps[m][:], eh_sb[NDh * j + NDh - 1][:, ts(m, P)],
                    y_tiles[NDh - 1][:],
                    start=False, stop=(j == 1))

        def emit_attn_half(h, j, z_ps, v_l, layer):
            s_tiles = []
            emit_s_rows(h, j, 0, NT, s_tiles)
            emit_ayz(h, j, z_ps, v_l, s_tiles)

        def emit_z_allreduce(h, z_ps, layer, nsplit=1):
            # Evict z (bf16), transpose to [T-half, D] packed layout, DMA to
            # DRAM, AllReduce over the core pair, DMA the sum back. With
            # nsplit>1 the message is split into independent ARs so the
            # consumer can pipeline against them (used for the final layer).
            z_sb = [zpool.tile([P, HW], bf16, tag=f"zsb{m}",
                               name=f"zsb{m}_{layer}_{h}") for m in range(ND)]
            for m in range(ND):
                nc.vector.tensor_copy(z_sb[m][:], z_ps[m][:])
            zq = zpool.tile([P, T], bf16, tag="zq", name=f"zq{layer}_{h}")
            for mm in range(NHB):
                for kd in range(ND):
                    tzp = psW.tile([P, P], bf16, tag="psW", name="tzp")
                    nc.tensor.transpose(
                        tzp[:], z_sb[kd][:, ts(mm, P)], identb[:])
                    dst = zq[:, mm * D + kd * P:mm * D + (kd + 1) * P]
                    if kd == 0:
                        nc.scalar.copy(dst, tzp[:])
                    else:
                        nc.vector.tensor_copy(dst, tzp[:])
            zrs = []
            w = T // nsplit
            for s in range(nsplit):
                zin = dpool.tile([P, w], bf16, tag=f"zin{h}_{s}",
                                 name=f"zin{h}_{s}_{layer}")
                zout = dpool.tile([P, w], bf16, tag=f"zout{h}_{s}",
                                  name=f"zout{h}_{s}_{layer}")
                nc.sync.dma_start(zin[:], zq[:, s * w:(s + 1) * w])
                nc.gpsimd.collective_compute(
                    "AllReduce", mybir.AluOpType.add,
                    ins=[zin.opt()], outs=[zout.opt()],
                    replica_groups=rg)
                zr = zpool.tile([P, w], bf16, tag=f"zr{h}_{s}",
                                name=f"zr{h}_{s}_{layer}")
                nc.sync.dma_start(zr[:], zout[:])
                zrs.append(zr)
            return zrs[0] if nsplit == 1 else zrs

        # ---- embedding: v = LN(onehot @ emb_w) ----
        for m in range(NT):
            eps_t = psW.tile([P, D], f32, tag="psW", name="embps")
            for k in range(ND):
                nc.tensor.matmul(eps_t[:], oh_sb[k][:, ts(m, P)], ew_sb[k][:],
                                 start=(k == 0), stop=(k == ND - 1))
            emb_t = lnpool.tile([P, D], f32, tag="w", name="embt")
            nc.scalar.copy(emb_t[:], eps_t[:])
            layer_norm(emb_t[:], v_cur[m][:])
            transpose_v(m)

        # ---- layers, software-pipelined across the per-half AllReduces ----
        # Invariant entering layer l: x/RoPE for half 0 of layer l are
        # emitted; for l>0 the AR for half 1 of layer l-1 is in flight and
        # its LN update has NOT been emitted yet. S(half0) rows 0..3 (which
        # need only half-0 RoPE) are emitted before that LN so the PE has
        # AR-independent work while the collective + LN chain drain.
        emit_x_half(0)
        zr_pend = None  # half-1 AR result of the previous layer
        for layer in range(L):
            s00, s01 = [], []
            emit_s_rows(0, 0, 0, NHB, s00)
            emit_s_rows(0, 1, 0, NHB, s01)
            if zr_pend is not None:
                emit_ln_half(1, zr_pend)  # waits on the in-flight AR
            v_l = list(v_cur)  # layer l's v is complete only after the LN
            emit_x_half(1)
            emit_s_rows(0, 0, NHB, NT, s00)
            z_ps = [psZ.tile([P, HW], f32, tag="psZ", name=f"zps{m}")
                    for m in range(ND)]
            emit_ayz(0, 0, z_ps, v_l, s00)
            emit_s_rows(0, 1, NHB, NT, s01)
            emit_ayz(0, 1, z_ps, v_l, s01)
            zr0 = emit_z_allreduce(0, z_ps, layer)
            z_ps = [psZ.tile([P, HW], f32, tag="psZ", name=f"zps{m}b")
                    for m in range(ND)]
            emit_attn_half(1, 0, z_ps, v_l, layer)
            # AR(half0) has landed behind half1's work; vT refresh is
            # deferred to the layer end so the PE never waits on this LN.
            emit_ln_half(0, zr0, with_tv=False)
            emit_attn_half(1, 1, z_ps, v_l, layer)
            if layer < L - 1:
                zr_pend = emit_z_allreduce(1, z_ps, layer)
                for mm in range(NHB):
                    transpose_v(mm)
                emit_x_half(0)
            else:
                zr_q = emit_z_allreduce(1, z_ps, layer)

        # ---- readout, pipelined against the final AR quarters ----
        def emit_readout_blocks(ms):
            for m in ms:
                rps = psW.tile([P, V], f32, tag="psW", name="rps")
                for k in range(ND):
                    nc.tensor.matmul(rps[:], vT_sb[k][:, ts(m, P)], ro_sb[k][:],
                                     start=(k == 0), stop=(k == ND - 1))
                o_sb = lnpool.tile([P, V], f32, tag="o", name=f"o{m}")
                nc.scalar.copy(o_sb[:], rps[:])
                nc.sync.dma_start(d_out[ts(m, P), :], o_sb[:])

        for mm in range(NHB):
            transpose_v(mm)
        emit_readout_blocks(range(0, 4))
        emit_ln_blocks([4, 5], zr_q, 0)
        for m in (4, 5):
            transpose_v(m)
        emit_readout_blocks([4, 5])
        emit_ln_blocks([6, 7], zr_q, 2 * D)
        for m in (6, 7):
            transpose_v(m)
        emit_readout_blocks([6, 7])

    nc.compile()
    return nc


def _get_program():
    if "nc" not in _CACHE:
        _CACHE["nc"] = _build_program()
    return _CACHE["nc"]


def _rope_tables():
    inv = (1.0 / (10000.0 ** (np.arange(0, Dh, 2, dtype=np.float32) / Dh)))
    tt = np.arange(T, dtype=np.float32)
    freqs = np.outer(tt, inv).astype(np.float32)  # [T, Dh/2]
    cosT = np.ascontiguousarray(np.cos(freqs).T, dtype=np.float32)
    sinT = np.ascontiguousarray(np.sin(freqs).T, dtype=np.float32)
    return cosT, sinT


def kernel(**inputs):
    global LAST_RESULT
    import ml_dtypes
    from concourse import bass_utils

    bf = ml_dtypes.bfloat16
    tokens = np.asarray(inputs["tokens"])
    emb_w = np.ascontiguousarray(inputs["emb_w"], dtype=np.float32)
    E = np.ascontiguousarray(inputs["E"], dtype=np.float32)
    Dx = np.ascontiguousarray(inputs["Dx"], dtype=np.float32)
    Dy = np.ascontiguousarray(inputs["Dy"], dtype=np.float32)
    readout = np.ascontiguousarray(inputs["readout"], dtype=np.float32)

    cosT, sinT = _rope_tables()

    in_maps = []
    for c in range(NCORES):
        b, hp = c // 2, c % 2
        oh = np.zeros((V, T), dtype=bf)
        oh[np.asarray(tokens[b], dtype=np.int64), np.arange(T)] = 1.0
        in_maps.append({
            "onehotT": oh,
            "emb_w": emb_w.astype(bf),
            "dx": np.ascontiguousarray(
                Dx[2 * hp:2 * hp + 2].reshape(2 * D, Dh).astype(bf)),
            "dy": np.ascontiguousarray(
                Dy[2 * hp:2 * hp + 2].reshape(2 * D, Dh).astype(bf)),
            "eh": np.ascontiguousarray(
                E[2 * hp * Dh:(2 * hp + 2) * Dh].astype(bf)),
            "cosT": cosT.astype(bf),
            "sinT": sinT.astype(bf),
            "readout": readout.astype(bf),
        })

    nc = _get_program()
    res = bass_utils.run_bass_kernel_spmd(
        nc, in_maps, core_ids=list(range(NCORES)),
        trace=bool(int(os.environ.get("KERNEL_TRACE", "0"))))
    LAST_RESULT = res
    out = np.stack([res.results[2 * b]["out"] for b in range(B)], axis=0)
    return out
